# revision 1
# baseline (speedup 1.0000x reference)
"""Trainium2 Bass kernel for BalancedConformationalConsistencyLoss.

Strategy (segment/fragment parallelism, 8 cores):
  * Host sorts nodes by fragment; 32 fragments per core (snake-deal by size),
    bin-packed into 9 strips of 128 slots so no fragment straddles a strip.
  * Device (per core, SPMD): encoder MLPs in feature-major (D on partitions)
    layout; per-node L2 norms via ones-matmul + DRAM bounce; normalized
    features (bf16); per-strip 128x128 gram blocks on the tensor engine;
    masked pair reductions via scalar_tensor_tensor/activation accumulators;
    sums via one-hot matmuls; per-core partial loss -> [1,3] output.
  * Host combines 8 partials (sum + the two global Frobenius sqrts).

All pairwise math uses the identity  sum_pm (S-t)^2 = A - 2tB + t^2*npairs
with A = sum pm*S^2, B = sum pm*S, so no (S-t) intermediates are built.
"""
import numpy as np
from contextlib import ExitStack

# ---------------- problem constants (hardcoded per contract) ----------------
N, D, NF = 8192, 256, 256
R = 0.3
BRICS = 0.4
AW = 0.6
VW = 0.2
CF = 0.1
NCORES = 8
NFC = NF // NCORES          # 32 fragments per core
P = 128                     # strip height
NSTRIP = 9                  # strips per core
M = NSTRIP * P              # 1152 padded slots per core
CH = 384                    # encoder column chunk
NCH = M // CH
LN2 = np.float32(np.log(2.0))

# Q column indices
(QB_SH, QA_SH, QA_UN, QC_UN, QB_DIR, QA_DIR,
 QSOFF, QVOFF, QNSQ_SH, QNSQ_UN, QNSQ_V, QJUNK) = range(12)
NQ = 12


# ============================ host-side prep ================================

def _assign_fragments(fid):
    counts = np.bincount(fid, minlength=NF)
    order = np.argsort(-counts, kind="stable")
    core_frags = [[] for _ in range(NCORES)]
    for i, f in enumerate(order):
        r = i // NCORES
        c = i % NCORES if r % 2 == 0 else NCORES - 1 - (i % NCORES)
        core_frags[c].append(int(f))
    layout = []
    for c in range(NCORES):
        frags = sorted(core_frags[c], key=lambda f: -counts[f])
        strips = [[] for _ in range(NSTRIP)]
        fill = np.zeros(NSTRIP, dtype=int)
        for f in frags:
            for s in range(NSTRIP):
                if fill[s] + counts[f] <= P:
                    strips[s].append(f)
                    fill[s] += counts[f]
                    break
            else:
                raise AssertionError(f"core {c}: fragment {f} does not fit")
        layout.append(strips)
    return layout, counts


def _build_core_meta(fid, atom_types, layout, counts):
    nodes_of = {f: np.nonzero(fid == f)[0] for f in range(NF)}
    metas = []
    for c in range(NCORES):
        slot_node = -np.ones(M, dtype=np.int64)
        slot_frag = -np.ones(M, dtype=np.int64)
        frag_global = []
        fl = 0
        for b in range(NSTRIP):
            pos = b * P
            for f in layout[c][b]:
                nn = nodes_of[f]
                slot_node[pos:pos + len(nn)] = nn
                slot_frag[pos:pos + len(nn)] = fl
                frag_global.append(f)
                pos += len(nn)
                fl += 1
        assert fl == NFC
        frag_global = np.array(frag_global, dtype=np.int64)
        real = slot_node >= 0

        pm3 = np.zeros((NSTRIP, P, 3 * P), dtype=np.float32)
        tm = np.zeros((NSTRIP, P, P), dtype=np.float32)
        amat = np.zeros((NSTRIP, P, NFC), dtype=np.float32)
        t2pm_frag = np.zeros(NFC, dtype=np.float32)
        for b in range(NSTRIP):
            sf = slot_frag[b * P:(b + 1) * P]
            sn = slot_node[b * P:(b + 1) * P]
            rr = sf >= 0
            same = (sf[:, None] == sf[None, :]) & rr[:, None] & rr[None, :]
            upper = np.triu(np.ones((P, P), dtype=bool), k=1)
            pmb = (same & upper).astype(np.float32)
            pm3[b] = np.tile(pmb, (1, 3))
            at = np.where(rr, atom_types[np.where(rr, sn, 0)], -1)
            tgt = np.where(at[:, None] == at[None, :], 0.3, 0.1).astype(np.float32)
            tm[b] = tgt * pmb
            for p in range(P):
                if rr[p]:
                    amat[b, p, sf[p]] = 1.0
            for fl_ in np.unique(sf[rr]):
                sel = sf == fl_
                t2pm_frag[fl_] += float(((tgt * pmb) ** 2)[np.ix_(sel, sel)].sum())

        cnt = counts[frag_global].astype(np.float32)
        valid = (cnt >= 2.0).astype(np.float32)
        pairs = cnt * (cnt - 1.0) * 0.5
        safe_c = np.maximum(cnt, 1.0)
        safe_p = np.maximum(pairs, 1.0)

        cvec = np.zeros((NFC, NQ), dtype=np.float32)
        t_sh = np.float32(BRICS)
        t_dir = np.float32(BRICS - 0.2)
        cvec[:, QB_SH] = -2.0 * t_sh * AW / safe_p
        cvec[:, QA_SH] = AW / safe_p
        cvec[:, QA_UN] = (1.0 - AW) / safe_p
        cvec[:, QC_UN] = -2.0 * (1.0 - AW) / safe_p
        cvec[:, QB_DIR] = -2.0 * t_dir * VW / (3.0 * safe_p)
        cvec[:, QA_DIR] = VW / (3.0 * safe_p)
        cvec[:, QSOFF] = -2.0 * AW / (safe_c * safe_c)
        cvec[:, QVOFF] = -2.0 * VW / (safe_c * safe_c)
        cvec[:, QNSQ_SH] = AW * (1.0 / safe_c - 1.0 / (safe_c * safe_c))
        cvec[:, QNSQ_V] = VW * (1.0 / safe_c - 1.0 / (safe_c * safe_c))
        cvec[:, QJUNK] = (AW * t_sh * t_sh * pairs / safe_p
                          + (1.0 - AW) * t2pm_frag / safe_p
                          + VW * t_dir * t_dir * pairs / safe_p)
        cvec *= valid[:, None]

        metas.append(dict(slot_node=slot_node, real=real, pm3=pm3, tm=tm,
                          amat=amat, cvec=cvec))
    return metas


def _shard_inputs(inputs):
    fid = np.asarray(inputs["fragment_ids"]).astype(np.int64)
    at = np.asarray(inputs["atom_types"]).astype(np.int64)
    layout, counts = _assign_fragments(fid)
    metas = _build_core_meta(fid, at, layout, counts)

    W1 = np.asarray(inputs["W1"], np.float32)
    W2 = np.asarray(inputs["W2"], np.float32)
    Wd1 = np.asarray(inputs["Wd1"], np.float32)
    Wd2 = np.asarray(inputs["Wd2"], np.float32)
    Wv1 = np.asarray(inputs["Wv1"], np.float32)
    Wv2 = np.asarray(inputs["Wv2"], np.float32)
    w1c = np.ascontiguousarray(np.concatenate([R * W1, (1.0 - R) * W1], axis=0), np.float32)
    wv1c = np.ascontiguousarray(np.concatenate([R * Wv1, (1.0 - R) * Wv1], axis=0), np.float32)
    b2p = (np.asarray(inputs["b2"], np.float32) - LN2 * W2.sum(axis=0)).astype(np.float32)
    bd2p = (np.asarray(inputs["bd2"], np.float32) - LN2 * Wd2.sum(axis=0)).astype(np.float32)
    bv2p = (-LN2 * Wv2.sum(axis=0)).astype(np.float32)

    ss = np.asarray(inputs["scalar_short"], np.float32)
    sl = np.asarray(inputs["scalar_long"], np.float32)
    vs = np.asarray(inputs["vector_short"], np.float32)
    vl = np.asarray(inputs["vector_long"], np.float32)

    in_maps = []
    for c in range(NCORES):
        m = metas[c]
        idx = np.where(m["real"], m["slot_node"], 0)
        rmask2 = m["real"][:, None]

        def take2(x):
            g = x[idx] * rmask2
            return np.ascontiguousarray(g.T, dtype=np.float32)

        def take3(x):
            g = x[idx] * m["real"][:, None, None]
            return np.ascontiguousarray(
                g.transpose(1, 2, 0).reshape(3 * D, M), dtype=np.float32)

        in_maps.append({
            "x_s": take2(ss), "x_l": take2(sl),
            "v_s": take3(vs), "v_l": take3(vl),
            "w1c": w1c, "w2": W2, "wd1": Wd1, "wd2": Wd2,
            "wv1c": wv1c, "wv2": Wv2,
            "b1": np.asarray(inputs["b1"], np.float32).reshape(D, 1),
            "b2p": b2p.reshape(D, 1),
            "bd1": np.asarray(inputs["bd1"], np.float32).reshape(2 * D, 1),
            "bd2p": bd2p.reshape(2 * D, 1),
            "bv2p": bv2p.reshape(D, 1),
            "pm3": m["pm3"], "tm": m["tm"],
            "amat": m["amat"], "cvec": m["cvec"],
            "ident3": np.ascontiguousarray(np.tile(np.eye(P, dtype=np.float32), (1, 3))),
        })
    n_valid = float((counts >= 2).sum())
    return in_maps, n_valid


def _combine(fins, n_valid):
    loss = float(sum(float(f[0]) for f in fins))
    ssq_sh = float(sum(float(f[1]) for f in fins))
    ssq_un = float(sum(float(f[2]) for f in fins))
    l2 = 0.01 * (np.sqrt(ssq_sh) + np.sqrt(ssq_un))
    if n_valid > 0:
        return np.float32(CF * (loss + n_valid * l2) / max(n_valid, 1.0))
    return np.float32(0.0)


# ============================ device program ================================

_NC_CACHE = {}


def build_nc():
    import os
    PH = int(os.environ.get("BCCL_PHASES", "9"))
    if "nc" in _NC_CACHE:
        return _NC_CACHE["nc"]
    import concourse.bass as bass
    import concourse.bacc as bacc
    import concourse.mybir as mybir
    import concourse.tile as tile

    F32 = mybir.dt.float32
    BF16 = mybir.dt.bfloat16
    AF = mybir.ActivationFunctionType
    ALU = mybir.AluOpType

    nc = bacc.Bacc("TRN2", target_bir_lowering=False, debug=False)

    d = {}
    for name, shape in [
        ("x_s", [D, M]), ("x_l", [D, M]), ("v_s", [3 * D, M]), ("v_l", [3 * D, M]),
        ("w1c", [2 * D, D]), ("w2", [D, D]), ("wd1", [D, 2 * D]), ("wd2", [2 * D, 2 * D]),
        ("wv1c", [2 * D, D]), ("wv2", [D, D]),
        ("b1", [D, 1]), ("b2p", [D, 1]), ("bd1", [2 * D, 1]), ("bd2p", [2 * D, 1]),
        ("bv2p", [D, 1]),
        ("pm3", [NSTRIP, P, 3 * P]), ("tm", [NSTRIP, P, P]),
        ("amat", [NSTRIP, P, NFC]), ("cvec", [NFC, NQ]),
        ("ident3", [P, 3 * P]),
    ]:
        d[name] = nc.dram_tensor(name, shape, F32, kind="ExternalInput").ap()
    d_out = nc.dram_tensor("out", [1, 3], F32, kind="ExternalOutput").ap()
    d_inv = nc.dram_tensor("inv_scratch", [5, M], F32).ap()

    with tile.TileContext(nc) as tc, ExitStack() as ctx:
        wpool = ctx.enter_context(tc.tile_pool(name="w", bufs=1))
        feat = ctx.enter_context(tc.tile_pool(name="feat", bufs=1))
        xin = ctx.enter_context(tc.tile_pool(name="xin", bufs=2))
        small = ctx.enter_context(tc.tile_pool(name="small", bufs=1))
        rowp = ctx.enter_context(tc.tile_pool(name="rowp", bufs=2))
        junkp = ctx.enter_context(tc.tile_pool(name="junk", bufs=3))
        maskp = ctx.enter_context(tc.tile_pool(name="mask", bufs=2))
        psp = ctx.enter_context(tc.tile_pool(name="ps", bufs=6, space="PSUM"))
        pseg = ctx.enter_context(tc.tile_pool(name="pseg", bufs=1, space="PSUM"))

        # ---- constants / weights ----
        def load_w(name, kt, cols):
            ts_ = []
            for k in range(kt):
                w = wpool.tile([P, cols], F32, tag=f"{name}{k}", name=f"{name}{k}")
                nc.sync.dma_start(out=w, in_=d[name][k * P:(k + 1) * P, :])
                ts_.append(w)
            return ts_

        w1c_t = load_w("w1c", 4, D)
        w2_t = load_w("w2", 2, D)
        wd1_t = load_w("wd1", 2, 2 * D)
        wd2_t = load_w("wd2", 4, 2 * D)
        wv1c_t = load_w("wv1c", 4, D)
        wv2_t = load_w("wv2", 2, D)

        def load_bias(name, mt):
            b = wpool.tile([P, mt], F32, tag=f"b_{name}", name=f"b_{name}")
            nc.sync.dma_start(out=b, in_=d[name].rearrange("(m p) o -> p (m o)", p=P))
            return b

        b1_sb = load_bias("b1", 2)
        b2p_sb = load_bias("b2p", 2)
        bd1_sb = load_bias("bd1", 4)
        bd2p_sb = load_bias("bd2p", 4)
        bv2p_sb = load_bias("bv2p", 2)

        ones1 = wpool.tile([1, P], F32, tag="ones1", name="ones1")
        nc.vector.memset(ones1, 1.0)
        ones32 = wpool.tile([NFC, 1], F32, tag="ones32", name="ones32")
        nc.vector.memset(ones32, 1.0)

        amat_sb = []
        for b in range(NSTRIP):
            a = wpool.tile([P, NFC], F32, tag=f"amat{b}", name=f"amat{b}")
            nc.sync.dma_start(out=a, in_=d["amat"][b])
            amat_sb.append(a)
        cvec_sb = wpool.tile([NFC, NQ], F32, tag="cvec", name="cvec")
        nc.sync.dma_start(out=cvec_sb, in_=d["cvec"])

        # ---- persistent feature tiles ----
        sh_u = [feat.tile([P, M], F32, tag=f"sh_u{i}", name=f"sh_u{i}") for i in range(2)]
        un_u = [feat.tile([P, M], F32, tag=f"un_u{i}", name=f"un_u{i}") for i in range(2)]
        v_u = [feat.tile([P, M], F32, tag=f"v_u{i}", name=f"v_u{i}") for i in range(6)]
        sh_n = [feat.tile([P, M], BF16, tag=f"sh_n{i}", name=f"sh_n{i}") for i in range(2)]
        un_n = [feat.tile([P, M], BF16, tag=f"un_n{i}", name=f"un_n{i}") for i in range(2)]
        v_n = [feat.tile([P, M], BF16, tag=f"v_n{i}", name=f"v_n{i}") for i in range(6)]

        # ---- encoder ----
        for c in range(NCH):
            cs = c * CH
            csl = slice(cs, cs + CH)

            def load_x(name, kt, tagp, shared_mod=None, vbufs=2):
                ts_ = []
                for k in range(kt):
                    tg = f"{tagp}{k % shared_mod}" if shared_mod else f"{tagp}{k}"
                    t = xin.tile([P, CH], F32, tag=tg, name=f"{tagp}{k}", bufs=vbufs)
                    nc.sync.dma_start(out=t, in_=d[name][k * P:(k + 1) * P, csl])
                    ts_.append(t)
                return ts_

            xs_t = load_x("x_s", 2, "xs")
            xl_t = load_x("x_l", 2, "xl")
            vs_t = load_x("v_s", 6, "vs", shared_mod=2, vbufs=3)
            vl_t = load_x("v_l", 6, "vl", shared_mod=2, vbufs=3)

            def layer(w_tiles, rhs_tiles, mt, evac):
                outs = []
                for m_ in range(mt):
                    pt = psp.tile([P, 512], F32, tag="ps", name="ps")
                    kt = len(rhs_tiles)
                    for k in range(kt):
                        nc.tensor.matmul(pt[:, 0:CH],
                                         w_tiles[k][:, m_ * P:(m_ + 1) * P],
                                         rhs_tiles[k],
                                         start=(k == 0), stop=(k == kt - 1))
                    outs.append(evac(m_, pt))
                return outs

            def act_evac(tag, bias_sb):
                # softplus(z+b) = ln(exp(z+b) + 1); exp & ln share one ACT table set
                def f(m_, pt):
                    e = xin.tile([P, CH], F32, tag=f"e{tag}{m_}", name=f"e{tag}{m_}", bufs=1)
                    if bias_sb is None:
                        nc.scalar.activation(e, pt[:, 0:CH], AF.Exp)
                    else:
                        nc.scalar.activation(e, pt[:, 0:CH], AF.Exp,
                                             bias=bias_sb[:, m_:m_ + 1])
                    t = xin.tile([P, CH], F32, tag=f"{tag}{m_}", name=f"{tag}{m_}", bufs=1)
                    nc.scalar.activation(t, e, AF.Ln, bias=1.0)
                    return t
                return f

            def add_evac(dst_tiles, bias_sb):
                def f(m_, pt):
                    nc.vector.tensor_scalar_add(dst_tiles[m_][:, csl], pt[:, 0:CH],
                                                bias_sb[:, m_:m_ + 1])
                    return None
                return f

            h1 = layer(w1c_t, xs_t + xl_t, 2, act_evac("h1", b1_sb))
            # s_inv with DVE evac into temp tiles
            s_ = []
            for m_ in range(2):
                pt = psp.tile([P, 512], F32, tag="ps", name="ps")
                for k in range(2):
                    nc.tensor.matmul(pt[:, 0:CH], w2_t[k][:, m_ * P:(m_ + 1) * P],
                                     h1[k], start=(k == 0), stop=(k == 1))
                t = xin.tile([P, CH], F32, tag=f"s{m_}", name=f"s{m_}", bufs=1)
                nc.vector.tensor_scalar_add(t, pt[:, 0:CH], b2p_sb[:, m_:m_ + 1])
                s_.append(t)
            hd = layer(wd1_t, s_, 4, act_evac("hd", bd1_sb))
            layer(wd2_t, hd, 4, add_evac(sh_u + un_u, bd2p_sb))
            for dd in range(3):
                rhs = [vs_t[2 * dd], vs_t[2 * dd + 1], vl_t[2 * dd], vl_t[2 * dd + 1]]
                v1 = layer(wv1c_t, rhs, 2, act_evac(f"v1_{dd}", None))
                layer(wv2_t, v1, 2, add_evac(v_u[2 * dd:2 * dd + 2], bv2p_sb))

        # ---- G1: unnormalized grams -> diag (per-node nsq) + off-diag sums ----
        if PH < 2:
            dbg = small.tile([1, 3], F32, tag="dbg", name="dbg")
            nc.vector.tensor_copy(dbg, sh_u[0][0:1, 0:3])
            nc.sync.dma_start(out=d_out, in_=dbg)
        if PH >= 2:
            ident3 = wpool.tile([P, 3 * P], F32, tag="ident3", name="ident3")
            nc.sync.dma_start(out=ident3, in_=d["ident3"])

            Q = small.tile([P, NSTRIP, NQ], F32, tag="Q", name="Q")
            nsqN = small.tile([P, NSTRIP, 5], F32, tag="nsqN", name="nsqN")

            pm3_sb = []
            for b in range(NSTRIP):
                t = wpool.tile([P, 3 * P], F32, tag=f"pm3_{b}", name=f"pm3_{b}")
                nc.sync.dma_start(out=t, in_=d["pm3"][b])
                pm3_sb.append(t)

            def gram(pt, tiles, ncol_tiles, b):
                bsl = slice(b * P, (b + 1) * P)
                for g in range(ncol_tiles):
                    for k in range(2):
                        nc.tensor.matmul(pt[:, g * P:(g + 1) * P],
                                         tiles[2 * g + k][:, bsl],
                                         tiles[2 * g + k][:, bsl],
                                         start=(k == 0), stop=(k == 1))
                return pt

            def msum(out_t, in0, in1, acc, width):
                # out = in0 * in1 ; acc[p] = row-sum(out)
                nc.vector.scalar_tensor_tensor(
                    out=out_t[:, 0:width], in0=in0, scalar=1.0, in1=in1,
                    op0=ALU.bypass, op1=ALU.mult, accum_out=acc)

            def sqsum(out_t, in_, acc, width):
                # out = in_^2 ; acc[p] = row-sum(out)
                nc.scalar.activation(out_t[:, 0:width], in_, AF.Square,
                                     accum_out=acc)

            for b in range(NSTRIP):
                gu_sh = psp.tile([P, 512], F32, tag="ps", name="gu_sh")
                gram(gu_sh, sh_u, 1, b)
                gu_un = psp.tile([P, 512], F32, tag="ps", name="gu_un")
                gram(gu_un, un_u, 1, b)
                gu_d = psp.tile([P, 512], F32, tag="ps", name="gu_d")
                gram(gu_d, v_u, 3, b)

                j5 = junkp.tile([P, P], F32, tag="jk1", name="j5")
                msum(j5, gu_sh[:, 0:P], pm3_sb[b][:, 0:P], Q[:, b, QSOFF:QSOFF + 1], P)
                j6 = junkp.tile([P, 3 * P], F32, tag="jk3", name="j6")
                msum(j6, gu_d[:, 0:3 * P], pm3_sb[b][:, 0:3 * P], Q[:, b, QVOFF:QVOFF + 1], 3 * P)
                jd0 = junkp.tile([P, P], F32, tag="jk1", name="jd0")
                msum(jd0, gu_sh[:, 0:P], ident3[:, 0:P], nsqN[:, b, 0:1], P)
                jd1 = junkp.tile([P, P], F32, tag="jk1", name="jd1")
                msum(jd1, gu_un[:, 0:P], ident3[:, 0:P], nsqN[:, b, 1:2], P)
                for dd in range(3):
                    jdv = junkp.tile([P, P], F32, tag="jk1", name=f"jdv{dd}")
                    msum(jdv, gu_d[:, dd * P:(dd + 1) * P], ident3[:, 0:P],
                         nsqN[:, b, 2 + dd:3 + dd], P)

            # Q nsq columns
            nc.vector.tensor_copy(Q[:, :, QNSQ_SH], nsqN[:, :, 0])
            nc.vector.tensor_copy(Q[:, :, QNSQ_UN], nsqN[:, :, 1])
            nc.vector.tensor_add(Q[:, :, QNSQ_V], nsqN[:, :, 2], nsqN[:, :, 3])
            nc.vector.tensor_add(Q[:, :, QNSQ_V], Q[:, :, QNSQ_V], nsqN[:, :, 4])

        if PH == 2:
            dbg = small.tile([1, 3], F32, tag="dbg", name="dbg")
            nc.vector.tensor_copy(dbg, Q[0:1, 0, 0:3])
            nc.sync.dma_start(out=d_out, in_=dbg)
        if PH >= 3:
            # ---- inverse norms (node-major) -> DRAM -> rows -> bcast -> normalize ----
            sq = small.tile([P, NSTRIP * 5], F32, tag="sqn", name="sqn")
            nc.vector.tensor_scalar_max(sq, nsqN.rearrange("p b s -> p (b s)"), 1e-24)
            nc.scalar.activation(sq, sq, AF.Ln)
            nc.scalar.activation(sq, sq, AF.Exp, scale=0.5)
            invN = small.tile([P, NSTRIP, 5], F32, tag="invN", name="invN")
            nc.vector.reciprocal(invN.rearrange("p b s -> p (b s)"), sq)
            for s in range(5):
                nc.sync.dma_start(out=d_inv[s:s + 1, :].rearrange("o (b p) -> p b o", p=P),
                                  in_=invN[:, :, s])

            sets_u = [sh_u, un_u, v_u[0:2], v_u[2:4], v_u[4:6]]
            sets_n = [sh_n, un_n, v_n[0:2], v_n[2:4], v_n[4:6]]
            for s in range(5):
                irow = rowp.tile([1, M], F32, tag="invrow", name="invrow")
                nc.sync.dma_start(out=irow, in_=d_inv[s:s + 1, :])
                for ci in range(NCH):
                    bc = psp.tile([P, 512], F32, tag="ps", name="bc")
                    nc.tensor.matmul(bc[:, 0:CH], ones1,
                                     irow[0:1, ci * CH:(ci + 1) * CH],
                                     start=True, stop=True)
                    for k in range(2):
                        nc.vector.tensor_mul(
                            sets_n[s][k][:, ci * CH:(ci + 1) * CH],
                            sets_u[s][k][:, ci * CH:(ci + 1) * CH],
                            bc[:, 0:CH])

        if PH == 3:
            dbg = small.tile([1, 3], F32, tag="dbg", name="dbg")
            nc.vector.tensor_copy(dbg, sh_n[0][0:1, 0:3])
            nc.sync.dma_start(out=d_out, in_=dbg)
        if PH >= 4:
            # ---- G2: normalized grams + masked pair-mse reductions ----
            for b in range(NSTRIP):
                g_sh = psp.tile([P, 512], F32, tag="ps", name="g_sh")
                gram(g_sh, sh_n, 1, b)
                g_un = psp.tile([P, 512], F32, tag="ps", name="g_un")
                gram(g_un, un_n, 1, b)
                g_d = psp.tile([P, 512], F32, tag="ps", name="g_d")
                gram(g_d, v_n, 3, b)

                tmb = maskp.tile([P, P], F32, tag="tm", name="tmb")
                nc.sync.dma_start(out=tmb, in_=d["tm"][b])

                spm = junkp.tile([P, P], F32, tag="spm", name="spm", bufs=2)
                msum(spm, g_sh[:, 0:P], pm3_sb[b][:, 0:P], Q[:, b, QB_SH:QB_SH + 1], P)
                j1 = junkp.tile([P, P], F32, tag="jk1", name="j1")
                sqsum(j1, spm[:, 0:P], Q[:, b, QA_SH:QA_SH + 1], P)

                upm = junkp.tile([P, P], F32, tag="upm", name="upm", bufs=2)
                msum(upm, g_un[:, 0:P], pm3_sb[b][:, 0:P], Q[:, b, QJUNK:QJUNK + 1], P)
                j2 = junkp.tile([P, P], F32, tag="jk1", name="j2")
                sqsum(j2, upm[:, 0:P], Q[:, b, QA_UN:QA_UN + 1], P)
                j3 = junkp.tile([P, P], F32, tag="jk1", name="j3")
                msum(j3, g_un[:, 0:P], tmb[:, 0:P], Q[:, b, QC_UN:QC_UN + 1], P)

                dpm = junkp.tile([P, 3 * P], F32, tag="dpm", name="dpm", bufs=2)
                msum(dpm, g_d[:, 0:3 * P], pm3_sb[b][:, 0:3 * P], Q[:, b, QB_DIR:QB_DIR + 1], 3 * P)
                j4 = junkp.tile([P, 3 * P], F32, tag="jk3", name="j4")
                sqsum(j4, dpm[:, 0:3 * P], Q[:, b, QA_DIR:QA_DIR + 1], 3 * P)

        if PH >= 4:
            # ---- segment reduction + final combine ----
            seg_ps = pseg.tile([NFC, NQ], F32, tag="seg", name="seg")
            for b in range(NSTRIP):
                nc.tensor.matmul(seg_ps, amat_sb[b], Q[:, b, :],
                                 start=(b == 0), stop=(b == NSTRIP - 1))
            segs = small.tile([NFC, NQ], F32, tag="segs", name="segs")
            nc.vector.tensor_copy(segs, seg_ps)

            acc = small.tile([NFC, 3], F32, tag="acc", name="acc")
            nc.vector.tensor_scalar_mul(acc[:, 0:1], segs[:, 0:1], cvec_sb[:, 0:1])
            for q in range(1, 11):
                nc.vector.scalar_tensor_tensor(
                    out=acc[:, 0:1], in0=segs[:, q:q + 1], scalar=cvec_sb[:, q:q + 1],
                    in1=acc[:, 0:1], op0=ALU.mult, op1=ALU.add)
            nc.vector.tensor_add(acc[:, 0:1], acc[:, 0:1], cvec_sb[:, QJUNK:QJUNK + 1])
            nc.vector.tensor_copy(acc[:, 1:2], segs[:, QNSQ_SH:QNSQ_SH + 1])
            nc.vector.tensor_copy(acc[:, 2:3], segs[:, QNSQ_UN:QNSQ_UN + 1])

            fin_ps = pseg.tile([1, 3], F32, tag="fin", name="fin")
            nc.tensor.matmul(fin_ps, ones32, acc, start=True, stop=True)
            fin_sb = small.tile([1, 3], F32, tag="fin_sb", name="fin_sb")
            nc.vector.tensor_copy(fin_sb, fin_ps)
            nc.sync.dma_start(out=d_out, in_=fin_sb)

    nc.compile()
    _NC_CACHE["nc"] = nc
    return nc


# ============================== entry point =================================

def kernel(**inputs) -> np.ndarray:
    from concourse.bass_utils import run_bass_kernel_spmd

    in_maps, n_valid = _shard_inputs(inputs)
    nc = build_nc()
    res = run_bass_kernel_spmd(nc, in_maps, core_ids=list(range(NCORES)))
    fins = [r["out"].reshape(3) for r in res.results]
    return _combine(fins, n_valid)



# revision 4
# speedup vs baseline: 1.6228x; 1.6228x over previous
"""Trainium2 Bass kernel for BalancedConformationalConsistencyLoss.

Strategy (segment/fragment parallelism, 8 cores):
  * Host sorts nodes by fragment; 32 fragments per core (snake-deal by size),
    bin-packed into 9 strips of 128 slots so no fragment straddles a strip.
  * Device (per core, SPMD): encoder MLPs in feature-major (D on partitions)
    layout, all matmuls bf16; Softplus evacs on the scalar engine (single
    table set); per-strip 128x128 gram blocks on the tensor engine;
    masked pair reductions via scalar_tensor_tensor/activation accumulators;
    inverse norms via Ln/Exp(-0.5) with a small DRAM bounce to node-major;
    sums via one-hot matmuls; per-core partial loss -> [1,3] output.
  * Host combines 8 partials (sum + the two global Frobenius sqrts).

All pairwise math uses the identity  sum_pm (S-t)^2 = A - 2tB + t^2*npairs
with A = sum pm*S^2, B = sum pm*S, so no (S-t) intermediates are built.
"""
import numpy as np
import ml_dtypes
from contextlib import ExitStack

BF = ml_dtypes.bfloat16

# ---------------- problem constants (hardcoded per contract) ----------------
N, D, NF = 8192, 256, 256
R = 0.3
BRICS = 0.4
AW = 0.6
VW = 0.2
CF = 0.1
NCORES = 8
NFC = NF // NCORES          # 32 fragments per core
P = 128                     # strip height
NSTRIP = 9                  # strips per core
M = NSTRIP * P              # 1152 padded slots per core
CH = 384                    # encoder column chunk
NCH = M // CH
LN2 = np.float32(np.log(2.0))

# Q column indices
(QB_SH, QA_SH, QA_UN, QC_UN, QB_DIR, QA_DIR,
 QSOFF, QVOFF, QNSQ_SH, QNSQ_UN, QNSQ_V, QJUNK) = range(12)
NQ = 12


# ============================ host-side prep ================================

def _assign_fragments(fid):
    counts = np.bincount(fid, minlength=NF)
    order = np.argsort(-counts, kind="stable")
    core_frags = [[] for _ in range(NCORES)]
    for i, f in enumerate(order):
        r = i // NCORES
        c = i % NCORES if r % 2 == 0 else NCORES - 1 - (i % NCORES)
        core_frags[c].append(int(f))
    layout = []
    for c in range(NCORES):
        frags = sorted(core_frags[c], key=lambda f: -counts[f])
        strips = [[] for _ in range(NSTRIP)]
        fill = np.zeros(NSTRIP, dtype=int)
        for f in frags:
            for s in range(NSTRIP):
                if fill[s] + counts[f] <= P:
                    strips[s].append(f)
                    fill[s] += counts[f]
                    break
            else:
                raise AssertionError(f"core {c}: fragment {f} does not fit")
        layout.append(strips)
    return layout, counts


def _build_core_meta(fid, atom_types, layout, counts):
    nodes_of = {f: np.nonzero(fid == f)[0] for f in range(NF)}
    metas = []
    for c in range(NCORES):
        slot_node = -np.ones(M, dtype=np.int64)
        slot_frag = -np.ones(M, dtype=np.int64)
        frag_global = []
        fl = 0
        for b in range(NSTRIP):
            pos = b * P
            for f in layout[c][b]:
                nn = nodes_of[f]
                slot_node[pos:pos + len(nn)] = nn
                slot_frag[pos:pos + len(nn)] = fl
                frag_global.append(f)
                pos += len(nn)
                fl += 1
        assert fl == NFC
        frag_global = np.array(frag_global, dtype=np.int64)
        real = slot_node >= 0

        pm3 = np.zeros((NSTRIP, P, 3 * P), dtype=np.float32)
        tm = np.zeros((NSTRIP, P, P), dtype=np.float32)
        amat = np.zeros((NSTRIP, P, NFC), dtype=np.float32)
        t2pm_frag = np.zeros(NFC, dtype=np.float32)
        for b in range(NSTRIP):
            sf = slot_frag[b * P:(b + 1) * P]
            sn = slot_node[b * P:(b + 1) * P]
            rr = sf >= 0
            same = (sf[:, None] == sf[None, :]) & rr[:, None] & rr[None, :]
            upper = np.triu(np.ones((P, P), dtype=bool), k=1)
            pmb = (same & upper).astype(np.float32)
            pm3[b] = np.tile(pmb, (1, 3))
            at = np.where(rr, atom_types[np.where(rr, sn, 0)], -1)
            tgt = np.where(at[:, None] == at[None, :], 0.3, 0.1).astype(np.float32)
            tm[b] = tgt * pmb
            for p in range(P):
                if rr[p]:
                    amat[b, p, sf[p]] = 1.0
            for fl_ in np.unique(sf[rr]):
                sel = sf == fl_
                t2pm_frag[fl_] += float(((tgt * pmb) ** 2)[np.ix_(sel, sel)].sum())

        cnt = counts[frag_global].astype(np.float32)
        valid = (cnt >= 2.0).astype(np.float32)
        pairs = cnt * (cnt - 1.0) * 0.5
        safe_c = np.maximum(cnt, 1.0)
        safe_p = np.maximum(pairs, 1.0)

        cvec = np.zeros((NFC, NQ), dtype=np.float32)
        t_sh = np.float32(BRICS)
        t_dir = np.float32(BRICS - 0.2)
        cvec[:, QB_SH] = -2.0 * t_sh * AW / safe_p
        cvec[:, QA_SH] = AW / safe_p
        cvec[:, QA_UN] = (1.0 - AW) / safe_p
        cvec[:, QC_UN] = -2.0 * (1.0 - AW) / safe_p
        cvec[:, QB_DIR] = -2.0 * t_dir * VW / (3.0 * safe_p)
        cvec[:, QA_DIR] = VW / (3.0 * safe_p)
        cvec[:, QSOFF] = -2.0 * AW / (safe_c * safe_c)
        cvec[:, QVOFF] = -2.0 * VW / (safe_c * safe_c)
        cvec[:, QNSQ_SH] = AW * (1.0 / safe_c - 1.0 / (safe_c * safe_c))
        cvec[:, QNSQ_V] = VW * (1.0 / safe_c - 1.0 / (safe_c * safe_c))
        cvec[:, QJUNK] = (AW * t_sh * t_sh * pairs / safe_p
                          + (1.0 - AW) * t2pm_frag / safe_p
                          + VW * t_dir * t_dir * pairs / safe_p)
        cvec *= valid[:, None]

        metas.append(dict(slot_node=slot_node, real=real, pm3=pm3, tm=tm,
                          amat=amat, cvec=cvec))
    return metas


def _shard_inputs(inputs):
    fid = np.asarray(inputs["fragment_ids"]).astype(np.int64)
    at = np.asarray(inputs["atom_types"]).astype(np.int64)
    layout, counts = _assign_fragments(fid)
    metas = _build_core_meta(fid, at, layout, counts)

    W1 = np.asarray(inputs["W1"], np.float32)
    W2 = np.asarray(inputs["W2"], np.float32)
    Wd1 = np.asarray(inputs["Wd1"], np.float32)
    Wd2 = np.asarray(inputs["Wd2"], np.float32)
    Wv1 = np.asarray(inputs["Wv1"], np.float32)
    Wv2 = np.asarray(inputs["Wv2"], np.float32)
    w1c = np.ascontiguousarray(np.concatenate([R * W1, (1.0 - R) * W1], axis=0), BF)
    wv1c = np.ascontiguousarray(np.concatenate([R * Wv1, (1.0 - R) * Wv1], axis=0), BF)
    b2p = (np.asarray(inputs["b2"], np.float32) - LN2 * W2.sum(axis=0)).astype(np.float32)
    bd2p = (np.asarray(inputs["bd2"], np.float32) - LN2 * Wd2.sum(axis=0)).astype(np.float32)
    bv2p = (-LN2 * Wv2.sum(axis=0)).astype(np.float32)

    ss = np.asarray(inputs["scalar_short"], np.float32)
    sl = np.asarray(inputs["scalar_long"], np.float32)
    vs = np.asarray(inputs["vector_short"], np.float32)
    vl = np.asarray(inputs["vector_long"], np.float32)

    in_maps = []
    for c in range(NCORES):
        m = metas[c]
        idx = np.where(m["real"], m["slot_node"], 0)
        rmask2 = m["real"][:, None]

        def take2(x):
            g = x[idx] * rmask2
            return np.ascontiguousarray(g.T.astype(BF))

        def take3(x):
            g = x[idx] * m["real"][:, None, None]
            return np.ascontiguousarray(
                g.transpose(1, 2, 0).reshape(3 * D, M).astype(BF))

        in_maps.append({
            "x_s": take2(ss), "x_l": take2(sl),
            "v_s": take3(vs), "v_l": take3(vl),
            "w1c": w1c, "w2": W2.astype(BF), "wd1": Wd1.astype(BF),
            "wd2": Wd2.astype(BF), "wv1c": wv1c, "wv2": Wv2.astype(BF),
            "b1": np.asarray(inputs["b1"], np.float32).reshape(D, 1),
            "b2p": b2p.reshape(D, 1),
            "bd1": np.asarray(inputs["bd1"], np.float32).reshape(2 * D, 1),
            "bd2p": bd2p.reshape(2 * D, 1),
            "bv2p": bv2p.reshape(D, 1),
            "pm3": m["pm3"].astype(BF), "tm": m["tm"].astype(BF),
            "amat": m["amat"], "cvec": m["cvec"],
            "ident3": np.ascontiguousarray(
                np.tile(np.eye(P, dtype=np.float32), (1, 3)).astype(BF)),
        })
    n_valid = float((counts >= 2).sum())
    return in_maps, n_valid


def _combine(fins, n_valid):
    loss = float(sum(float(f[0]) for f in fins))
    ssq_sh = float(sum(float(f[1]) for f in fins))
    ssq_un = float(sum(float(f[2]) for f in fins))
    l2 = 0.01 * (np.sqrt(ssq_sh) + np.sqrt(ssq_un))
    if n_valid > 0:
        return np.float32(CF * (loss + n_valid * l2) / max(n_valid, 1.0))
    return np.float32(0.0)


# ============================ device program ================================

_NC_CACHE = {}


def build_nc():
    if "nc" in _NC_CACHE:
        return _NC_CACHE["nc"]
    import concourse.bass as bass
    import concourse.bacc as bacc
    import concourse.mybir as mybir
    import concourse.tile as tile

    F32 = mybir.dt.float32
    BF16 = mybir.dt.bfloat16
    AF = mybir.ActivationFunctionType
    ALU = mybir.AluOpType

    nc = bacc.Bacc("TRN2", target_bir_lowering=False, debug=False)

    d = {}
    for name, shape, dt_ in [
        ("x_s", [D, M], BF16), ("x_l", [D, M], BF16),
        ("v_s", [3 * D, M], BF16), ("v_l", [3 * D, M], BF16),
        ("w1c", [2 * D, D], BF16), ("w2", [D, D], BF16),
        ("wd1", [D, 2 * D], BF16), ("wd2", [2 * D, 2 * D], BF16),
        ("wv1c", [2 * D, D], BF16), ("wv2", [D, D], BF16),
        ("b1", [D, 1], F32), ("b2p", [D, 1], F32), ("bd1", [2 * D, 1], F32),
        ("bd2p", [2 * D, 1], F32), ("bv2p", [D, 1], F32),
        ("pm3", [NSTRIP, P, 3 * P], BF16), ("tm", [NSTRIP, P, P], BF16),
        ("amat", [NSTRIP, P, NFC], F32), ("cvec", [NFC, NQ], F32),
        ("ident3", [P, 3 * P], BF16),
    ]:
        d[name] = nc.dram_tensor(name, shape, dt_, kind="ExternalInput").ap()
    d_out = nc.dram_tensor("out", [1, 3], F32, kind="ExternalOutput").ap()
    d_inv = nc.dram_tensor("inv_scratch", [5, M], BF16).ap()

    with tile.TileContext(nc) as tc, ExitStack() as ctx:
        wpool = ctx.enter_context(tc.tile_pool(name="w", bufs=1))
        feat = ctx.enter_context(tc.tile_pool(name="feat", bufs=1))
        xin = ctx.enter_context(tc.tile_pool(name="xin", bufs=2))
        small = ctx.enter_context(tc.tile_pool(name="small", bufs=1))
        rowp = ctx.enter_context(tc.tile_pool(name="rowp", bufs=2))
        junkp = ctx.enter_context(tc.tile_pool(name="junk", bufs=3))
        maskp = ctx.enter_context(tc.tile_pool(name="mask", bufs=2))
        psp = ctx.enter_context(tc.tile_pool(name="ps", bufs=6, space="PSUM"))
        pseg = ctx.enter_context(tc.tile_pool(name="pseg", bufs=1, space="PSUM"))

        # ---- constants / weights ----
        def load_w(name, kt, cols):
            ts_ = []
            for k in range(kt):
                w = wpool.tile([P, cols], BF16, tag=f"{name}{k}", name=f"{name}{k}")
                nc.sync.dma_start(out=w, in_=d[name][k * P:(k + 1) * P, :])
                ts_.append(w)
            return ts_

        w1c_t = load_w("w1c", 4, D)
        w2_t = load_w("w2", 2, D)
        wd1_t = load_w("wd1", 2, 2 * D)
        wd2_t = load_w("wd2", 4, 2 * D)
        wv1c_t = load_w("wv1c", 4, D)
        wv2_t = load_w("wv2", 2, D)

        def load_bias(name, mt):
            b = wpool.tile([P, mt], F32, tag=f"b_{name}", name=f"b_{name}")
            nc.sync.dma_start(out=b, in_=d[name].rearrange("(m p) o -> p (m o)", p=P))
            return b

        b1_sb = load_bias("b1", 2)
        b2p_sb = load_bias("b2p", 2)
        bd1_sb = load_bias("bd1", 4)
        bd2p_sb = load_bias("bd2p", 4)
        bv2p_sb = load_bias("bv2p", 2)

        ones1 = wpool.tile([1, P], BF16, tag="ones1", name="ones1")
        nc.vector.memset(ones1, 1.0)
        ones32 = wpool.tile([NFC, 1], F32, tag="ones32", name="ones32")
        nc.vector.memset(ones32, 1.0)

        amat_sb = []
        for b in range(NSTRIP):
            a = wpool.tile([P, NFC], F32, tag=f"amat{b}", name=f"amat{b}")
            nc.sync.dma_start(out=a, in_=d["amat"][b])
            amat_sb.append(a)
        cvec_sb = wpool.tile([NFC, NQ], F32, tag="cvec", name="cvec")
        nc.sync.dma_start(out=cvec_sb, in_=d["cvec"])

        # ---- persistent feature tiles ----
        sh_u = [feat.tile([P, M], BF16, tag=f"sh_u{i}", name=f"sh_u{i}") for i in range(2)]
        un_u = [feat.tile([P, M], BF16, tag=f"un_u{i}", name=f"un_u{i}") for i in range(2)]
        v_u = [feat.tile([P, M], BF16, tag=f"v_u{i}", name=f"v_u{i}") for i in range(6)]
        sh_n = [feat.tile([P, M], BF16, tag=f"sh_n{i}", name=f"sh_n{i}") for i in range(2)]
        un_n = [feat.tile([P, M], BF16, tag=f"un_n{i}", name=f"un_n{i}") for i in range(2)]
        v_n = [feat.tile([P, M], BF16, tag=f"v_n{i}", name=f"v_n{i}") for i in range(6)]

        # ---- encoder ----
        # Activations are batched BY FUNCTION across all chunks so the scalar
        # engine loads each activation table once per phase (5 loads total)
        # instead of thrashing Exp<->Ln per tile.  softplus(z+b) is computed
        # as e = Exp(z+b) [phase X], t = Ln(e + 1) [phase L, one wide inst
        # per chunk]; the -ln2 shift is folded into the next layer's bias.
        def mm_layer(pt, w_tiles, rhs_tiles, m_):
            kt = len(rhs_tiles)
            for k in range(kt):
                nc.tensor.matmul(pt[:, 0:CH],
                                 w_tiles[k][:, m_ * P:(m_ + 1) * P],
                                 rhs_tiles[k],
                                 start=(k == 0), stop=(k == kt - 1))

        def exp_to(dst_wide, j, pt, bias_col):
            if bias_col is None:
                nc.scalar.activation(dst_wide[:, j * CH:(j + 1) * CH],
                                     pt[:, 0:CH], AF.Exp)
            else:
                nc.scalar.activation(dst_wide[:, j * CH:(j + 1) * CH],
                                     pt[:, 0:CH], AF.Exp, bias=bias_col)

        xs_t, xl_t, e_h1, t_h1 = [], [], [], []
        for c in range(NCH):
            csl = slice(c * CH, (c + 1) * CH)

            def load_x(name, kt, tagp):
                # one DMA per tensor per chunk: [kt*128, CH] -> [128, kt*CH]
                t = xin.tile([P, kt * CH], BF16, tag=f"{tagp}{c}", name=f"{tagp}{c}",
                             bufs=1)
                nc.sync.dma_start(
                    out=t.rearrange("p (k m) -> p k m", k=kt),
                    in_=d[name].rearrange("(k p) m -> p k m", p=P)[:, :, csl])
                return [t[:, k * CH:(k + 1) * CH] for k in range(kt)]

            xs_t.append(load_x("x_s", 2, "xs"))
            xl_t.append(load_x("x_l", 2, "xl"))

            # h1 = Exp(x@W1c + b1) per m-tile
            ew = xin.tile([P, 2 * CH], BF16, tag=f"eh1_{c}", name=f"eh1_{c}", bufs=1)
            for m_ in range(2):
                pt = psp.tile([P, 512], F32, tag="ps", name="ps")
                mm_layer(pt, w1c_t, xs_t[c] + xl_t[c], m_)
                exp_to(ew, m_, pt, b1_sb[:, m_:m_ + 1])
            e_h1.append(ew)
        for c in range(NCH):
            tw = xin.tile([P, 2 * CH], BF16, tag=f"th1_{c}", name=f"th1_{c}", bufs=1)
            nc.scalar.activation(tw, e_h1[c], AF.Ln, bias=1.0)
            t_h1.append([tw[:, k * CH:(k + 1) * CH] for k in range(2)])
        s_all = []
        for c in range(NCH):
            s_ = []
            for m_ in range(2):
                pt = psp.tile([P, 512], F32, tag="ps", name="ps")
                mm_layer(pt, w2_t, t_h1[c], m_)
                t = xin.tile([P, CH], BF16, tag=f"s{m_}_{c}", name=f"s{m_}_{c}", bufs=1)
                nc.vector.tensor_scalar_add(t, pt[:, 0:CH], b2p_sb[:, m_:m_ + 1])
                s_.append(t)
            s_all.append(s_)

        # phase B: hd (4 tiles) + v1 (6 tiles) exps, then one wide Ln per chunk
        vs_t, vl_t, e_B = [], [], []
        for c in range(NCH):
            csl = slice(c * CH, (c + 1) * CH)

            def load_v(name, tagp):
                t = xin.tile([P, 6 * CH], BF16, tag=f"{tagp}{c}", name=f"{tagp}{c}",
                             bufs=1)
                nc.sync.dma_start(
                    out=t.rearrange("p (k m) -> p k m", k=6),
                    in_=d[name].rearrange("(k p) m -> p k m", p=P)[:, :, csl])
                return [t[:, k * CH:(k + 1) * CH] for k in range(6)]

            vs_t.append(load_v("v_s", "vs"))
            vl_t.append(load_v("v_l", "vl"))

            ew = xin.tile([P, 10 * CH], BF16, tag=f"eB_{c}", name=f"eB_{c}", bufs=1)
            for m_ in range(4):
                pt = psp.tile([P, 512], F32, tag="ps", name="ps")
                mm_layer(pt, wd1_t, s_all[c], m_)
                exp_to(ew, m_, pt, bd1_sb[:, m_:m_ + 1])
            for dd in range(3):
                rhs = [vs_t[c][2 * dd], vs_t[c][2 * dd + 1],
                       vl_t[c][2 * dd], vl_t[c][2 * dd + 1]]
                for m_ in range(2):
                    pt = psp.tile([P, 512], F32, tag="ps", name="ps")
                    mm_layer(pt, wv1c_t, rhs, m_)
                    exp_to(ew, 4 + 2 * dd + m_, pt, None)
            e_B.append(ew)
        t_B = []
        for c in range(NCH):
            tw = xin.tile([P, 10 * CH], BF16, tag=f"tB_{c}", name=f"tB_{c}", bufs=1)
            nc.scalar.activation(tw, e_B[c], AF.Ln, bias=1.0)
            t_B.append(tw)
        for c in range(NCH):
            csl = slice(c * CH, (c + 1) * CH)
            hd = [t_B[c][:, k * CH:(k + 1) * CH] for k in range(4)]
            for m_ in range(4):
                pt = psp.tile([P, 512], F32, tag="ps", name="ps")
                mm_layer(pt, wd2_t, hd, m_)
                nc.vector.tensor_scalar_add((sh_u + un_u)[m_][:, csl], pt[:, 0:CH],
                                            bd2p_sb[:, m_:m_ + 1])
            for dd in range(3):
                v1 = [t_B[c][:, (4 + 2 * dd + k) * CH:(5 + 2 * dd + k) * CH]
                      for k in range(2)]
                for m_ in range(2):
                    pt = psp.tile([P, 512], F32, tag="ps", name="ps")
                    mm_layer(pt, wv2_t, v1, m_)
                    nc.vector.tensor_scalar_add(v_u[2 * dd + m_][:, csl],
                                                pt[:, 0:CH], bv2p_sb[:, m_:m_ + 1])

        # ---- G1: unnormalized grams -> diag (per-node nsq) + off-diag sums ----
        ident3 = wpool.tile([P, 3 * P], BF16, tag="ident3", name="ident3")
        nc.sync.dma_start(out=ident3, in_=d["ident3"])

        Q = small.tile([P, NSTRIP, NQ], F32, tag="Q", name="Q")
        nsqN = small.tile([P, NSTRIP, 5], F32, tag="nsqN", name="nsqN")

        pm3_sb = []
        for b in range(NSTRIP):
            t = wpool.tile([P, 3 * P], BF16, tag=f"pm3_{b}", name=f"pm3_{b}")
            nc.sync.dma_start(out=t, in_=d["pm3"][b])
            pm3_sb.append(t)

        def gram(pt, tiles, ncol_tiles, b):
            bsl = slice(b * P, (b + 1) * P)
            for g in range(ncol_tiles):
                for k in range(2):
                    nc.tensor.matmul(pt[:, g * P:(g + 1) * P],
                                     tiles[2 * g + k][:, bsl],
                                     tiles[2 * g + k][:, bsl],
                                     start=(k == 0), stop=(k == 1))
            return pt

        def msum(out_t, in0, in1, acc, width):
            # out = in0 * in1 ; acc[p] = row-sum(out)
            nc.vector.scalar_tensor_tensor(
                out=out_t[:, 0:width], in0=in0, scalar=1.0, in1=in1,
                op0=ALU.bypass, op1=ALU.mult, accum_out=acc)

        def sqsum(out_t, in_, acc, width):
            # out = in_^2 ; acc[p] = row-sum(out)
            nc.scalar.activation(out_t[:, 0:width], in_, AF.Square,
                                 accum_out=acc)

        for b in range(NSTRIP):
            gu_sh = psp.tile([P, 512], F32, tag="ps", name="gu_sh")
            gram(gu_sh, sh_u, 1, b)
            gu_un = psp.tile([P, 512], F32, tag="ps", name="gu_un")
            gram(gu_un, un_u, 1, b)
            gu_d = psp.tile([P, 512], F32, tag="ps", name="gu_d")
            gram(gu_d, v_u, 3, b)

            j5 = junkp.tile([P, P], BF16, tag="jk1", name="j5")
            msum(j5, gu_sh[:, 0:P], pm3_sb[b][:, 0:P], Q[:, b, QSOFF:QSOFF + 1], P)
            j6 = junkp.tile([P, 3 * P], BF16, tag="jk3", name="j6")
            msum(j6, gu_d[:, 0:3 * P], pm3_sb[b][:, 0:3 * P], Q[:, b, QVOFF:QVOFF + 1], 3 * P)
            jd0 = junkp.tile([P, P], BF16, tag="jk1", name="jd0")
            msum(jd0, gu_sh[:, 0:P], ident3[:, 0:P], nsqN[:, b, 0:1], P)
            jd1 = junkp.tile([P, P], BF16, tag="jk1", name="jd1")
            msum(jd1, gu_un[:, 0:P], ident3[:, 0:P], nsqN[:, b, 1:2], P)
            for dd in range(3):
                jdv = junkp.tile([P, P], BF16, tag="jk1", name=f"jdv{dd}")
                msum(jdv, gu_d[:, dd * P:(dd + 1) * P], ident3[:, 0:P],
                     nsqN[:, b, 2 + dd:3 + dd], P)

        # Q nsq columns
        nc.vector.tensor_copy(Q[:, :, QNSQ_SH], nsqN[:, :, 0])
        nc.vector.tensor_copy(Q[:, :, QNSQ_UN], nsqN[:, :, 1])
        nc.vector.tensor_add(Q[:, :, QNSQ_V], nsqN[:, :, 2], nsqN[:, :, 3])
        nc.vector.tensor_add(Q[:, :, QNSQ_V], Q[:, :, QNSQ_V], nsqN[:, :, 4])

        # ---- inverse norms: inv = (nsq + eps)^(-1/2) via Ln/Exp (one table
        # set), node-major bounce through DRAM -> rows -> bcast -> normalize --
        eps_sb = small.tile([P, 1], F32, tag="eps", name="eps")
        nc.vector.memset(eps_sb, 1e-20)
        lnn = small.tile([P, NSTRIP * 5], F32, tag="lnn", name="lnn")
        nc.scalar.activation(lnn, nsqN.rearrange("p b s -> p (b s)"), AF.Ln,
                             bias=eps_sb[:, 0:1])
        invN = small.tile([P, NSTRIP, 5], BF16, tag="invN", name="invN")
        nc.scalar.activation(invN.rearrange("p b s -> p (b s)"), lnn, AF.Exp,
                             scale=-0.5)
        for s in range(5):
            nc.sync.dma_start(out=d_inv[s:s + 1, :].rearrange("o (b p) -> p b o", p=P),
                              in_=invN[:, :, s])

        sets_u = [sh_u, un_u, v_u[0:2], v_u[2:4], v_u[4:6]]
        sets_n = [sh_n, un_n, v_n[0:2], v_n[2:4], v_n[4:6]]
        for s in range(5):
            irow = rowp.tile([1, M], BF16, tag="invrow", name="invrow")
            nc.sync.dma_start(out=irow, in_=d_inv[s:s + 1, :])
            for ci in range(NCH):
                bc = psp.tile([P, 512], F32, tag="ps", name="bc")
                nc.tensor.matmul(bc[:, 0:CH], ones1,
                                 irow[0:1, ci * CH:(ci + 1) * CH],
                                 start=True, stop=True)
                for k in range(2):
                    nc.vector.tensor_mul(
                        sets_n[s][k][:, ci * CH:(ci + 1) * CH],
                        sets_u[s][k][:, ci * CH:(ci + 1) * CH],
                        bc[:, 0:CH])

        # ---- G2: normalized grams + masked pair-mse reductions ----
        for b in range(NSTRIP):
            g_sh = psp.tile([P, 512], F32, tag="ps", name="g_sh")
            gram(g_sh, sh_n, 1, b)
            g_un = psp.tile([P, 512], F32, tag="ps", name="g_un")
            gram(g_un, un_n, 1, b)
            g_d = psp.tile([P, 512], F32, tag="ps", name="g_d")
            gram(g_d, v_n, 3, b)

            tmb = maskp.tile([P, P], BF16, tag="tm", name="tmb")
            nc.sync.dma_start(out=tmb, in_=d["tm"][b])

            spm = junkp.tile([P, P], BF16, tag="spm", name="spm", bufs=2)
            msum(spm, g_sh[:, 0:P], pm3_sb[b][:, 0:P], Q[:, b, QB_SH:QB_SH + 1], P)
            j1 = junkp.tile([P, P], BF16, tag="jk1", name="j1")
            sqsum(j1, spm[:, 0:P], Q[:, b, QA_SH:QA_SH + 1], P)

            upm = junkp.tile([P, P], BF16, tag="upm", name="upm", bufs=2)
            msum(upm, g_un[:, 0:P], pm3_sb[b][:, 0:P], Q[:, b, QJUNK:QJUNK + 1], P)
            j2 = junkp.tile([P, P], BF16, tag="jk1", name="j2")
            sqsum(j2, upm[:, 0:P], Q[:, b, QA_UN:QA_UN + 1], P)
            j3 = junkp.tile([P, P], BF16, tag="jk1", name="j3")
            msum(j3, g_un[:, 0:P], tmb[:, 0:P], Q[:, b, QC_UN:QC_UN + 1], P)

            dpm = junkp.tile([P, 3 * P], BF16, tag="dpm", name="dpm", bufs=2)
            msum(dpm, g_d[:, 0:3 * P], pm3_sb[b][:, 0:3 * P], Q[:, b, QB_DIR:QB_DIR + 1], 3 * P)
            j4 = junkp.tile([P, 3 * P], BF16, tag="jk3", name="j4")
            sqsum(j4, dpm[:, 0:3 * P], Q[:, b, QA_DIR:QA_DIR + 1], 3 * P)

        # ---- segment reduction + final combine ----
        seg_ps = pseg.tile([NFC, NQ], F32, tag="seg", name="seg")
        for b in range(NSTRIP):
            nc.tensor.matmul(seg_ps, amat_sb[b], Q[:, b, :],
                             start=(b == 0), stop=(b == NSTRIP - 1))
        segs = small.tile([NFC, NQ], F32, tag="segs", name="segs")
        nc.vector.tensor_copy(segs, seg_ps)

        acc = small.tile([NFC, 3], F32, tag="acc", name="acc")
        nc.vector.tensor_scalar_mul(acc[:, 0:1], segs[:, 0:1], cvec_sb[:, 0:1])
        for q in range(1, 11):
            nc.vector.scalar_tensor_tensor(
                out=acc[:, 0:1], in0=segs[:, q:q + 1], scalar=cvec_sb[:, q:q + 1],
                in1=acc[:, 0:1], op0=ALU.mult, op1=ALU.add)
        nc.vector.tensor_add(acc[:, 0:1], acc[:, 0:1], cvec_sb[:, QJUNK:QJUNK + 1])
        nc.vector.tensor_copy(acc[:, 1:2], segs[:, QNSQ_SH:QNSQ_SH + 1])
        nc.vector.tensor_copy(acc[:, 2:3], segs[:, QNSQ_UN:QNSQ_UN + 1])

        fin_ps = pseg.tile([1, 3], F32, tag="fin", name="fin")
        nc.tensor.matmul(fin_ps, ones32, acc, start=True, stop=True)
        fin_sb = small.tile([1, 3], F32, tag="fin_sb", name="fin_sb")
        nc.vector.tensor_copy(fin_sb, fin_ps)
        nc.sync.dma_start(out=d_out, in_=fin_sb)

    nc.compile()
    _NC_CACHE["nc"] = nc
    return nc


# ============================== entry point =================================

def kernel(**inputs) -> np.ndarray:
    from concourse.bass_utils import run_bass_kernel_spmd

    in_maps, n_valid = _shard_inputs(inputs)
    nc = build_nc()
    res = run_bass_kernel_spmd(nc, in_maps, core_ids=list(range(NCORES)))
    fins = [r["out"].reshape(3) for r in res.results]
    return _combine(fins, n_valid)


# revision 17
# speedup vs baseline: 1.9375x; 1.1939x over previous
"""Trainium2 Bass kernel for BalancedConformationalConsistencyLoss.

Strategy (segment/fragment parallelism, 8 cores):
  * Host sorts nodes by fragment; 32 fragments per core (snake-deal by size),
    bin-packed into 9 strips of 128 slots so no fragment straddles a strip.
  * Device (per core, SPMD), all-bf16 matmul path:
      - encoder MLPs feature-major, softplus split into Exp/Ln phases batched
        by activation function (5 table loads total);
      - per-node squared norms via squared-features + per-strip one-column
        matmuls (node-major), inverse norms via Ln/Exp(-0.5);
      - norm rows bounced through DRAM to node-row layout, broadcast across
        partitions on GpSimd, features normalized on DVE;
      - per-strip 128x128 gram blocks (normalized + unnormalized) with masked
        pair reductions via scalar_tensor_tensor / activation accumulators;
      - per-fragment sums via one-hot matmuls; per-core partials -> [1,3].
  * Host combines 8 partials (sum + the two global Frobenius sqrts).

All pairwise math uses the identity  sum_pm (S-t)^2 = A - 2tB + t^2*npairs
with A = sum pm*S^2, B = sum pm*S, so no (S-t) intermediates are built.
"""
import numpy as np
import ml_dtypes
from contextlib import ExitStack

BF = ml_dtypes.bfloat16

# ---------------- problem constants (hardcoded per contract) ----------------
N, D, NF = 8192, 256, 256
R = 0.3
BRICS = 0.4
AW = 0.6
VW = 0.2
CF = 0.1
NCORES = 8
NFC = NF // NCORES          # 32 fragments per core
P = 128                     # strip height
NSTRIP = 9                  # strips per core
M = NSTRIP * P              # 1152 padded slots per core
CH = 384                    # encoder column chunk
NCH = M // CH
LN2 = np.float32(np.log(2.0))

# Q column indices (per-strip accumulator tile)
(QB_SH, QA_SH, QA_UN, QC_UN, QB_DIR, QA_DIR, QSOFF, QVOFF, QJK) = range(9)
NQ = 9
# cvec columns: 0..8 coeffs for Q cols, 9 = const, 10 = w_nsq_sh, 11 = w_nsq_v
NCV = 12

# weight-pack column offsets in the [128, 6144] packed weight tensor
_WOFF = {}
_off = 0
for _nm, _kt, _cols in [("w1c", 4, D), ("w2", 2, D), ("wd1", 2, 2 * D),
                        ("wd2", 4, 2 * D), ("wv1c", 4, D), ("wv2", 2, D)]:
    _WOFF[_nm] = (_off, _kt, _cols)
    _off += _kt * _cols
WCOLS = _off  # 6144


# ============================ host-side prep ================================

def _assign_fragments(fid):
    counts = np.bincount(fid, minlength=NF)
    order = np.argsort(-counts, kind="stable")
    core_frags = [[] for _ in range(NCORES)]
    for i, f in enumerate(order):
        r = i // NCORES
        c = i % NCORES if r % 2 == 0 else NCORES - 1 - (i % NCORES)
        core_frags[c].append(int(f))
    layout = []
    for c in range(NCORES):
        frags = sorted(core_frags[c], key=lambda f: -counts[f])
        strips = [[] for _ in range(NSTRIP)]
        fill = np.zeros(NSTRIP, dtype=int)
        for f in frags:
            for s in range(NSTRIP):
                if fill[s] + counts[f] <= P:
                    strips[s].append(f)
                    fill[s] += counts[f]
                    break
            else:
                raise AssertionError(f"core {c}: fragment {f} does not fit")
        layout.append(strips)
    return layout, counts


def _build_core_meta(fid, atom_types, layout, counts):
    nodes_of = {f: np.nonzero(fid == f)[0] for f in range(NF)}
    metas = []
    for c in range(NCORES):
        slot_node = -np.ones(M, dtype=np.int64)
        slot_frag = -np.ones(M, dtype=np.int64)
        frag_global = []
        fl = 0
        for b in range(NSTRIP):
            pos = b * P
            for f in layout[c][b]:
                nn = nodes_of[f]
                slot_node[pos:pos + len(nn)] = nn
                slot_frag[pos:pos + len(nn)] = fl
                frag_global.append(f)
                pos += len(nn)
                fl += 1
        assert fl == NFC
        frag_global = np.array(frag_global, dtype=np.int64)
        real = slot_node >= 0

        pm3 = np.zeros((NSTRIP, P, 3 * P), dtype=np.float32)
        tm = np.zeros((NSTRIP, P, P), dtype=np.float32)
        amat = np.zeros((NSTRIP, P, NFC), dtype=np.float32)
        t2pm_frag = np.zeros(NFC, dtype=np.float32)
        for b in range(NSTRIP):
            sf = slot_frag[b * P:(b + 1) * P]
            sn = slot_node[b * P:(b + 1) * P]
            rr = sf >= 0
            same = (sf[:, None] == sf[None, :]) & rr[:, None] & rr[None, :]
            upper = np.triu(np.ones((P, P), dtype=bool), k=1)
            pmb = (same & upper).astype(np.float32)
            pm3[b] = np.tile(pmb, (1, 3))
            at = np.where(rr, atom_types[np.where(rr, sn, 0)], -1)
            tgt = np.where(at[:, None] == at[None, :], 0.3, 0.1).astype(np.float32)
            tm[b] = tgt * pmb
            for p in range(P):
                if rr[p]:
                    amat[b, p, sf[p]] = 1.0
            for fl_ in np.unique(sf[rr]):
                sel = sf == fl_
                t2pm_frag[fl_] += float(((tgt * pmb) ** 2)[np.ix_(sel, sel)].sum())

        cnt = counts[frag_global].astype(np.float32)
        valid = (cnt >= 2.0).astype(np.float32)
        pairs = cnt * (cnt - 1.0) * 0.5
        safe_c = np.maximum(cnt, 1.0)
        safe_p = np.maximum(pairs, 1.0)

        t_sh = np.float32(BRICS)
        t_dir = np.float32(BRICS - 0.2)
        cvec = np.zeros((NFC, NCV), dtype=np.float32)
        cvec[:, QB_SH] = -2.0 * t_sh * AW / safe_p
        cvec[:, QA_SH] = AW / safe_p
        cvec[:, QA_UN] = (1.0 - AW) / safe_p
        cvec[:, QC_UN] = -2.0 * (1.0 - AW) / safe_p
        cvec[:, QB_DIR] = -2.0 * t_dir * VW / (3.0 * safe_p)
        cvec[:, QA_DIR] = VW / (3.0 * safe_p)
        cvec[:, QSOFF] = -2.0 * AW / (safe_c * safe_c)
        cvec[:, QVOFF] = -2.0 * VW / (safe_c * safe_c)
        cvec[:, QJK] = 0.0
        cvec[:, 9] = (AW * t_sh * t_sh * pairs / safe_p
                      + (1.0 - AW) * t2pm_frag / safe_p
                      + VW * t_dir * t_dir * pairs / safe_p)
        cvec[:, 10] = AW * (1.0 / safe_c - 1.0 / (safe_c * safe_c))
        cvec[:, 11] = VW * (1.0 / safe_c - 1.0 / (safe_c * safe_c))
        cvec[:, 0:10] *= valid[:, None]
        cvec[:, 10:12] *= valid[:, None]

        metas.append(dict(slot_node=slot_node, real=real, pm3=pm3, tm=tm,
                          amat=amat, cvec=cvec))
    return metas


def _pack_rows(x, kt):
    # [kt*128, cols] -> [128, kt*cols] so each partition's data is contiguous
    cols = x.shape[1]
    return np.ascontiguousarray(
        x.reshape(kt, P, cols).transpose(1, 0, 2).reshape(P, kt * cols))


def _shard_inputs(inputs):
    fid = np.asarray(inputs["fragment_ids"]).astype(np.int64)
    at = np.asarray(inputs["atom_types"]).astype(np.int64)
    layout, counts = _assign_fragments(fid)
    metas = _build_core_meta(fid, at, layout, counts)

    W1 = np.asarray(inputs["W1"], np.float32)
    W2 = np.asarray(inputs["W2"], np.float32)
    Wd1 = np.asarray(inputs["Wd1"], np.float32)
    Wd2 = np.asarray(inputs["Wd2"], np.float32)
    Wv1 = np.asarray(inputs["Wv1"], np.float32)
    Wv2 = np.asarray(inputs["Wv2"], np.float32)
    w1c = np.concatenate([R * W1, (1.0 - R) * W1], axis=0)
    wv1c = np.concatenate([R * Wv1, (1.0 - R) * Wv1], axis=0)
    wall = np.concatenate([
        _pack_rows(w1c, 4), _pack_rows(W2, 2), _pack_rows(Wd1, 2),
        _pack_rows(Wd2, 4), _pack_rows(wv1c, 4), _pack_rows(Wv2, 2)],
        axis=1).astype(BF)
    assert wall.shape == (P, WCOLS)

    b2p = (np.asarray(inputs["b2"], np.float32) - LN2 * W2.sum(axis=0))
    bd2p = (np.asarray(inputs["bd2"], np.float32) - LN2 * Wd2.sum(axis=0))
    bv2p = (-LN2 * Wv2.sum(axis=0))
    ball = np.concatenate([
        np.asarray(inputs["b1"], np.float32).reshape(2, P).T,
        b2p.reshape(2, P).T,
        np.asarray(inputs["bd1"], np.float32).reshape(4, P).T,
        bd2p.reshape(4, P).T,
        bv2p.reshape(2, P).T], axis=1).astype(np.float32)
    assert ball.shape == (P, 14)

    ss = np.asarray(inputs["scalar_short"], np.float32)
    sl = np.asarray(inputs["scalar_long"], np.float32)
    vs = np.asarray(inputs["vector_short"], np.float32)
    vl = np.asarray(inputs["vector_long"], np.float32)

    in_maps = []
    for c in range(NCORES):
        m = metas[c]
        idx = np.where(m["real"], m["slot_node"], 0)

        def take2(x):
            g = (x[idx] * m["real"][:, None]).astype(np.float32)
            return _pack_rows(np.ascontiguousarray(g.T), 2).astype(BF)

        def take3(x):
            g = (x[idx] * m["real"][:, None, None]).astype(np.float32)
            t = np.ascontiguousarray(g.transpose(1, 2, 0).reshape(3 * D, M))
            return _pack_rows(t, 6).astype(BF)

        in_maps.append({
            "x_s": take2(ss), "x_l": take2(sl),
            "v_s": take3(vs), "v_l": take3(vl),
            "wall": wall, "ball": ball,
            "pm3": np.ascontiguousarray(
                m["pm3"].transpose(1, 0, 2).reshape(P, NSTRIP * 3 * P)).astype(BF),
            "tm": np.ascontiguousarray(
                m["tm"].transpose(1, 0, 2).reshape(P, NSTRIP * P)).astype(BF),
            "amat": np.ascontiguousarray(
                m["amat"].transpose(1, 0, 2).reshape(P, NSTRIP * NFC)),
            "cvec": m["cvec"],
        })
    n_valid = float((counts >= 2).sum())
    return in_maps, n_valid


def _combine(fins, n_valid):
    loss = float(sum(float(f[0]) for f in fins))
    ssq_sh = float(sum(float(f[1]) for f in fins))
    ssq_un = float(sum(float(f[2]) for f in fins))
    l2 = 0.01 * (np.sqrt(ssq_sh) + np.sqrt(ssq_un))
    if n_valid > 0:
        return np.float32(CF * (loss + n_valid * l2) / max(n_valid, 1.0))
    return np.float32(0.0)


# ============================ device program ================================

_NC_CACHE = {}


def build_nc(debug_out=False):
    key = ("nc", debug_out)
    if key in _NC_CACHE:
        return _NC_CACHE[key]
    import concourse.bass as bass
    import concourse.bacc as bacc
    import concourse.mybir as mybir
    import concourse.tile as tile

    F32 = mybir.dt.float32
    BF16 = mybir.dt.bfloat16
    AF = mybir.ActivationFunctionType
    ALU = mybir.AluOpType

    nc = bacc.Bacc("TRN2", target_bir_lowering=False, debug=False)

    d = {}
    for name, shape, dt_ in [
        ("x_s", [P, 2 * M], BF16), ("x_l", [P, 2 * M], BF16),
        ("v_s", [P, 6 * M], BF16), ("v_l", [P, 6 * M], BF16),
        ("wall", [P, WCOLS], BF16), ("ball", [P, 14], F32),
        ("pm3", [P, NSTRIP * 3 * P], BF16), ("tm", [P, NSTRIP * P], BF16),
        ("amat", [P, NSTRIP * NFC], F32), ("cvec", [NFC, NCV], F32),
    ]:
        d[name] = nc.dram_tensor(name, shape, dt_, kind="ExternalInput").ap()
    d_out = nc.dram_tensor("out", [1, 3], F32, kind="ExternalOutput").ap()
    d_inv = nc.dram_tensor("inv_scratch", [5, M], BF16).ap()
    d_dbg = {}
    if debug_out:
        for nm, shape, dt_ in [("dbg_nsq", [P, 45], F32),
                               ("dbg_seg", [NFC, NQ], F32),
                               ("dbg_segn", [NFC, 5], F32),
                               ("dbg_irow", [1, 5 * M], F32)]:
            d_dbg[nm] = nc.dram_tensor(nm, shape, dt_, kind="ExternalOutput").ap()

    with tile.TileContext(nc) as tc, ExitStack() as ctx:
        wpool = ctx.enter_context(tc.tile_pool(name="w", bufs=1))
        feat = ctx.enter_context(tc.tile_pool(name="feat", bufs=1))
        xin = ctx.enter_context(tc.tile_pool(name="xin", bufs=2))
        small = ctx.enter_context(tc.tile_pool(name="small", bufs=1))
        rowp = ctx.enter_context(tc.tile_pool(name="rowp", bufs=2))
        junkp = ctx.enter_context(tc.tile_pool(name="junk", bufs=3))
        psp = ctx.enter_context(tc.tile_pool(name="ps", bufs=4, space="PSUM"))
        pnsq = ctx.enter_context(tc.tile_pool(name="pnsq", bufs=1, space="PSUM"))
        pseg = ctx.enter_context(tc.tile_pool(name="pseg", bufs=1, space="PSUM"))

        # ---- constants / weights (one DMA each) ----
        def ld(name, shape, dt_, pool=wpool):
            t = pool.tile(shape, dt_, tag=name, name=name)
            nc.sync.dma_start(out=t, in_=d[name])
            return t

        wall_sb = ld("wall", [P, WCOLS], BF16)
        ball_sb = ld("ball", [P, 14], F32)
        pm3_sb = ld("pm3", [P, NSTRIP * 3 * P], BF16)
        tm_sb = ld("tm", [P, NSTRIP * P], BF16)
        amat_sb = ld("amat", [P, NSTRIP * NFC], F32)
        cvec_sb = ld("cvec", [NFC, NCV], F32)
        xs_sb = ld("x_s", [P, 2 * M], BF16, pool=feat)
        xl_sb = ld("x_l", [P, 2 * M], BF16, pool=feat)
        vs_sb = ld("v_s", [P, 6 * M], BF16, pool=feat)
        vl_sb = ld("v_l", [P, 6 * M], BF16, pool=feat)

        def W(name, k):
            off, kt, cols = _WOFF[name]
            assert k < kt
            return wall_sb[:, off + k * cols: off + (k + 1) * cols]

        def bias(name, m_):
            offs = {"b1": 0, "b2p": 2, "bd1": 4, "bd2p": 8, "bv2p": 12}
            return ball_sb[:, offs[name] + m_: offs[name] + m_ + 1]

        ones_col = wpool.tile([P, 1], BF16, tag="ones_col", name="ones_col")
        nc.vector.memset(ones_col, 1.0)
        ones32 = wpool.tile([NFC, 1], F32, tag="ones32", name="ones32")
        nc.vector.memset(ones32, 1.0)
        eps_sb = wpool.tile([P, 1], F32, tag="eps", name="eps")
        nc.vector.memset(eps_sb, 1e-20)

        # ---- persistent per-chunk feature tiles ----
        # sets order: 0=sh 1=un 2=v0 3=v1 4=v2 ; each set has k-tiles 0,1
        u_t = [[[feat.tile([P, CH], BF16, tag=f"u{s}_{k}_{c}", name=f"u{s}_{k}_{c}")
                 for c in range(NCH)] for k in range(2)] for s in range(5)]
        n_t = [[[feat.tile([P, CH], BF16, tag=f"n{s}_{k}_{c}", name=f"n{s}_{k}_{c}")
                 for c in range(NCH)] for k in range(2)] for s in range(5)]

        nsq_ps = pnsq.tile([P, 45], F32, tag="nsq", name="nsq_ps")

        def mm_chunk(pt, w_name, rhs_list, m_):
            kt = len(rhs_list)
            for k in range(kt):
                nc.tensor.matmul(pt[:, 0:CH], W(w_name, k)[:, m_ * P:(m_ + 1) * P],
                                 rhs_list[k], start=(k == 0), stop=(k == kt - 1))

        def xsl(t, k, c):
            return t[:, k * M + c * CH: k * M + (c + 1) * CH]

        # ---- encoder phase A: h1 = softplus(x@W1c + b1), split Exp/Ln ----
        e_h1, t_h1, s_all = [], [], []
        for c in range(NCH):
            ew = xin.tile([P, 2 * CH], BF16, tag="eh1", name=f"eh1_{c}", bufs=2)
            for m_ in range(2):
                pt = psp.tile([P, 512], F32, tag="ps", name="ps")
                mm_chunk(pt, "w1c", [xsl(xs_sb, 0, c), xsl(xs_sb, 1, c),
                                     xsl(xl_sb, 0, c), xsl(xl_sb, 1, c)], m_)
                nc.scalar.activation(ew[:, m_ * CH:(m_ + 1) * CH], pt[:, 0:CH],
                                     AF.Exp, bias=bias("b1", m_))
            e_h1.append(ew)
        for c in range(NCH):
            tw = xin.tile([P, 2 * CH], BF16, tag="th1", name=f"th1_{c}", bufs=2)
            nc.scalar.activation(tw, e_h1[c], AF.Ln, bias=1.0)
            t_h1.append(tw)
        for c in range(NCH):
            s_ = []
            for m_ in range(2):
                pt = psp.tile([P, 512], F32, tag="ps", name="ps")
                mm_chunk(pt, "w2", [t_h1[c][:, 0:CH], t_h1[c][:, CH:2 * CH]], m_)
                t = xin.tile([P, CH], BF16, tag=f"s{m_}", name=f"s{m_}_{c}", bufs=2)
                nc.vector.tensor_scalar_add(t, pt[:, 0:CH], bias("b2p", m_))
                s_.append(t)
            s_all.append(s_)

        # ---- encoder phase B: hd + v1 exps, wide Ln, then dec/v2 evacs ----
        e_B, t_B = [], []
        for c in range(NCH):
            ew = xin.tile([P, 10 * CH], BF16, tag="eB", name=f"eB_{c}", bufs=2)
            for m_ in range(4):
                pt = psp.tile([P, 512], F32, tag="ps", name="ps")
                mm_chunk(pt, "wd1", s_all[c], m_)
                nc.scalar.activation(ew[:, m_ * CH:(m_ + 1) * CH], pt[:, 0:CH],
                                     AF.Exp, bias=bias("bd1", m_))
            for dd in range(3):
                rhs = [xsl(vs_sb, 2 * dd, c), xsl(vs_sb, 2 * dd + 1, c),
                       xsl(vl_sb, 2 * dd, c), xsl(vl_sb, 2 * dd + 1, c)]
                for m_ in range(2):
                    pt = psp.tile([P, 512], F32, tag="ps", name="ps")
                    mm_chunk(pt, "wv1c", rhs, m_)
                    j = 4 + 2 * dd + m_
                    nc.scalar.activation(ew[:, j * CH:(j + 1) * CH], pt[:, 0:CH],
                                         AF.Exp)
            e_B.append(ew)
        for c in range(NCH):
            tw = xin.tile([P, 10 * CH], BF16, tag="tB", name=f"tB_{c}", bufs=2)
            nc.scalar.activation(tw, e_B[c], AF.Ln, bias=1.0)
            t_B.append(tw)

        xsq_t = [[None] * 2 for _ in range(5)]

        def sq_and_nsq(s, k, c, src):
            # xsq = src^2; the one-column matmuls that reduce it to nsq are
            # emitted after the chunk's evacs so each column's two-matmul
            # accumulation group is contiguous
            t = feat.tile([P, CH], BF16, tag=f"xsq{s}_{k}", name=f"xsq{s}_{k}")
            if s < 2:
                nc.gpsimd.tensor_mul(t, src, src)
            else:
                nc.vector.tensor_mul(t, src, src)
            xsq_t[s][k] = t

        def nsq_mms(c):
            for s in range(5):
                for ls in range(3):
                    col = s * NSTRIP + 3 * c + ls
                    for k in range(2):
                        nc.tensor.matmul(
                            nsq_ps[:, col:col + 1],
                            xsq_t[s][k][:, ls * P:(ls + 1) * P], ones_col,
                            start=(k == 0), stop=(k == 1))

        for c in range(NCH):
            hd = [t_B[c][:, m_ * CH:(m_ + 1) * CH] for m_ in range(4)]
            for m_ in range(4):
                pt = psp.tile([P, 512], F32, tag="ps", name="ps")
                mm_chunk(pt, "wd2", hd, m_)
                s, k = (0, m_) if m_ < 2 else (1, m_ - 2)
                nc.scalar.activation(u_t[s][k][c], pt[:, 0:CH], AF.Identity,
                                     bias=bias("bd2p", m_))
                sq_and_nsq(s, k, c, u_t[s][k][c])
            for dd in range(3):
                v1 = [t_B[c][:, (4 + 2 * dd + k) * CH:(5 + 2 * dd + k) * CH]
                      for k in range(2)]
                for m_ in range(2):
                    pt = psp.tile([P, 512], F32, tag="ps", name="ps")
                    mm_chunk(pt, "wv2", v1, m_)
                    nc.vector.tensor_scalar_add(u_t[2 + dd][m_][c], pt[:, 0:CH],
                                                bias("bv2p", m_))
                    sq_and_nsq(2 + dd, m_, c, u_t[2 + dd][m_][c])
            nsq_mms(c)

        # ---- inverse norms: inv = (nsq+eps)^(-1/2), bounce to node rows ----
        nsq_sb = small.tile([P, 45], F32, tag="nsq_sb", name="nsq_sb")
        nc.vector.tensor_copy(nsq_sb, nsq_ps)
        lnn = small.tile([P, 45], F32, tag="lnn", name="lnn")
        nc.scalar.activation(lnn, nsq_ps, AF.Ln, bias=eps_sb[:, 0:1])
        invN = small.tile([P, 45], BF16, tag="invN", name="invN")
        nc.scalar.activation(invN, lnn, AF.Exp, scale=-0.5)
        for s in range(5):
            nc.sync.dma_start(
                out=d_inv[s:s + 1, :].rearrange("o (b p) -> p b o", p=P),
                in_=invN[:, s * NSTRIP:(s + 1) * NSTRIP])
        irow = rowp.tile([1, 5 * M], BF16, tag="irow", name="irow", bufs=1)
        nc.sync.dma_start(out=irow,
                          in_=d_inv.rearrange("(o s) m -> o (s m)", o=1))

        # ---- normalize: B_s = bcast(inv row) on GpSimd; n = u * B on DVE ----
        for s in range(5):
            B_s = rowp.tile([P, M], BF16, tag="Bb", name=f"B{s}", bufs=2)
            nc.gpsimd.partition_broadcast(B_s, irow[0:1, s * M:(s + 1) * M])
            for c in range(NCH):
                for k in range(2):
                    nc.vector.tensor_mul(n_t[s][k][c], u_t[s][k][c],
                                         B_s[:, c * CH:(c + 1) * CH])

        # ---- G phase: per-strip grams + masked pair reductions ----
        def msum(out_t, in0, in1, acc):
            nc.vector.scalar_tensor_tensor(
                out=out_t, in0=in0, scalar=1.0, in1=in1,
                op0=ALU.bypass, op1=ALU.mult, accum_out=acc)

        Q_b = []
        for b in range(NSTRIP):
            cb, ls = b // 3, (b % 3)
            lsl = slice(ls * P, (ls + 1) * P)
            pmb = pm3_sb[:, b * 3 * P: b * 3 * P + P]
            pm3b = pm3_sb[:, b * 3 * P: (b + 1) * 3 * P]
            tmb = tm_sb[:, b * P:(b + 1) * P]
            Q = small.tile([P, NQ], F32, tag=f"Q{b}", name=f"Q{b}")
            Q_b.append(Q)

            def gpair(pt, col0, tiles):
                for k in range(2):
                    nc.tensor.matmul(pt[:, col0:col0 + P], tiles[k][cb][:, lsl],
                                     tiles[k][cb][:, lsl],
                                     start=(k == 0), stop=(k == 1))

            # unnormalized grams (sh + 3 v dims) packed in one bank
            gu = psp.tile([P, 512], F32, tag="ps", name="gu")
            gpair(gu, 0, u_t[0])
            for dd in range(3):
                gpair(gu, P + dd * P, u_t[2 + dd])
            j5 = junkp.tile([P, P], BF16, tag="jk1", name="j5")
            msum(j5, gu[:, 0:P], pmb, Q[:, QSOFF:QSOFF + 1])
            j6 = junkp.tile([P, 3 * P], BF16, tag="jk3", name="j6")
            msum(j6, gu[:, P:4 * P], pm3b, Q[:, QVOFF:QVOFF + 1])

            # normalized grams: [sh | un] bank and [v0 | v1 | v2] bank
            gn = psp.tile([P, 512], F32, tag="ps", name="gn")
            gpair(gn, 0, n_t[0])
            gpair(gn, P, n_t[1])
            gv = psp.tile([P, 512], F32, tag="ps", name="gv")
            for dd in range(3):
                gpair(gv, dd * P, n_t[2 + dd])

            spm = junkp.tile([P, P], BF16, tag="spm", name="spm", bufs=2)
            msum(spm, gn[:, 0:P], pmb, Q[:, QB_SH:QB_SH + 1])
            nc.scalar.activation(junkp.tile([P, P], BF16, tag="jk1", name="j1"),
                                 spm, AF.Square, accum_out=Q[:, QA_SH:QA_SH + 1])
            upm = junkp.tile([P, P], BF16, tag="upm", name="upm", bufs=2)
            msum(upm, gn[:, P:2 * P], pmb, Q[:, QJK:QJK + 1])
            nc.scalar.activation(junkp.tile([P, P], BF16, tag="jk1", name="j2"),
                                 upm, AF.Square, accum_out=Q[:, QA_UN:QA_UN + 1])
            j3 = junkp.tile([P, P], BF16, tag="jk1", name="j3")
            msum(j3, gn[:, P:2 * P], tmb, Q[:, QC_UN:QC_UN + 1])

            dpm = junkp.tile([P, 3 * P], BF16, tag="dpm", name="dpm", bufs=2)
            msum(dpm, gv[:, 0:3 * P], pm3b, Q[:, QB_DIR:QB_DIR + 1])
            nc.scalar.activation(junkp.tile([P, 3 * P], BF16, tag="jk3", name="j4"),
                                 dpm, AF.Square, accum_out=Q[:, QA_DIR:QA_DIR + 1])

        # ---- segment reduction + final combine ----
        seg_ps = pseg.tile([NFC, NQ], F32, tag="seg", name="seg")
        segn_ps = pseg.tile([NFC, 5], F32, tag="segn", name="segn")
        nsq_v = nsq_sb.rearrange("p (s b) -> p s b", s=5)
        for b in range(NSTRIP):
            nc.tensor.matmul(seg_ps, amat_sb[:, b * NFC:(b + 1) * NFC], Q_b[b],
                             start=(b == 0), stop=(b == NSTRIP - 1))
        for b in range(NSTRIP):
            nc.tensor.matmul(segn_ps, amat_sb[:, b * NFC:(b + 1) * NFC],
                             nsq_v[:, :, b],
                             start=(b == 0), stop=(b == NSTRIP - 1))
        segs = small.tile([NFC, NQ], F32, tag="segs", name="segs")
        nc.vector.tensor_copy(segs, seg_ps)
        segn = small.tile([NFC, 5], F32, tag="segn_sb", name="segn_sb")
        nc.vector.tensor_copy(segn, segn_ps)

        acc = small.tile([NFC, 3], F32, tag="acc", name="acc")
        junkq = small.tile([NFC, NQ], F32, tag="junkq", name="junkq")
        # acc0 = sum_q cvec[q]*segs[q]  (rowwise dot via masked accumulate)
        nc.vector.scalar_tensor_tensor(
            out=junkq, in0=segs, scalar=1.0, in1=cvec_sb[:, 0:NQ],
            op0=ALU.bypass, op1=ALU.mult, accum_out=acc[:, 0:1])
        # + const + w_nsq_sh * segn_sh + w_nsq_v * (segn_v0+v1+v2)
        nc.vector.tensor_add(acc[:, 0:1], acc[:, 0:1], cvec_sb[:, 9:10])
        nc.vector.scalar_tensor_tensor(
            out=junkq[:, 0:1], in0=segn[:, 0:1], scalar=cvec_sb[:, 10:11],
            in1=acc[:, 0:1], op0=ALU.mult, op1=ALU.add, accum_out=None)
        nc.vector.tensor_copy(acc[:, 0:1], junkq[:, 0:1])
        vtot = small.tile([NFC, 1], F32, tag="vtot", name="vtot")
        nc.vector.tensor_add(vtot, segn[:, 2:3], segn[:, 3:4])
        nc.vector.tensor_add(vtot, vtot, segn[:, 4:5])
        nc.vector.scalar_tensor_tensor(
            out=junkq[:, 1:2], in0=vtot, scalar=cvec_sb[:, 11:12],
            in1=acc[:, 0:1], op0=ALU.mult, op1=ALU.add, accum_out=None)
        nc.vector.tensor_copy(acc[:, 0:1], junkq[:, 1:2])
        nc.vector.tensor_copy(acc[:, 1:2], segn[:, 0:1])
        nc.vector.tensor_copy(acc[:, 2:3], segn[:, 1:2])

        if debug_out:
            nc.sync.dma_start(out=d_dbg["dbg_nsq"], in_=nsq_sb)
            nc.sync.dma_start(out=d_dbg["dbg_seg"], in_=segs)
            nc.sync.dma_start(out=d_dbg["dbg_segn"], in_=segn)
            irow_f = rowp.tile([1, 5 * M], F32, tag="irow_f", name="irow_f", bufs=1)
            nc.vector.tensor_copy(irow_f, irow)
            nc.sync.dma_start(out=d_dbg["dbg_irow"], in_=irow_f)

        fin_ps = pseg.tile([1, 3], F32, tag="fin", name="fin")
        nc.tensor.matmul(fin_ps, ones32, acc, start=True, stop=True)
        fin_sb = small.tile([1, 3], F32, tag="fin_sb", name="fin_sb")
        nc.vector.tensor_copy(fin_sb, fin_ps)
        nc.sync.dma_start(out=d_out, in_=fin_sb)

    nc.compile()
    _NC_CACHE[key] = nc
    return nc


# ============================== entry point =================================

def kernel(**inputs) -> np.ndarray:
    from concourse.bass_utils import run_bass_kernel_spmd

    in_maps, n_valid = _shard_inputs(inputs)
    nc = build_nc()
    res = run_bass_kernel_spmd(nc, in_maps, core_ids=list(range(NCORES)))
    fins = [r["out"].reshape(3) for r in res.results]
    return _combine(fins, n_valid)


# revision 22
# speedup vs baseline: 2.4975x; 1.2890x over previous
"""Trainium2 Bass kernel for BalancedConformationalConsistencyLoss.

Strategy (segment/fragment parallelism, 8 cores):
  * Host sorts nodes by fragment; 32 fragments per core (snake-deal by size),
    bin-packed into 9 strips of 128 slots so no fragment straddles a strip.
  * Device (per core, SPMD), all-bf16 matmul path:
      - encoder MLPs feature-major, softplus split into Exp/Ln phases batched
        by activation function (5 table loads total);
      - per-node squared norms via squared-features + per-strip one-column
        matmuls (node-major), inverse norms via Ln/Exp(-0.5);
      - norm rows bounced through DRAM to node-row layout, broadcast across
        partitions on GpSimd, features normalized on DVE;
      - per-strip 128x128 gram blocks (normalized + unnormalized) with masked
        pair reductions via scalar_tensor_tensor / activation accumulators;
      - per-fragment sums via one-hot matmuls; per-core partials -> [1,3].
  * Host combines 8 partials (sum + the two global Frobenius sqrts).

All pairwise math uses the identity  sum_pm (S-t)^2 = A - 2tB + t^2*npairs
with A = sum pm*S^2, B = sum pm*S, so no (S-t) intermediates are built.
"""
import numpy as np
import ml_dtypes
from contextlib import ExitStack

BF = ml_dtypes.bfloat16

# ---------------- problem constants (hardcoded per contract) ----------------
N, D, NF = 8192, 256, 256
R = 0.3
BRICS = 0.4
AW = 0.6
VW = 0.2
CF = 0.1
NCORES = 8
NFC = NF // NCORES          # 32 fragments per core
P = 128                     # strip height
NSTRIP = 9                  # strips per core
M = NSTRIP * P              # 1152 padded slots per core
CH = 384                    # encoder column chunk
NCH = M // CH
LN2 = np.float32(np.log(2.0))

# Q column indices (per-strip accumulator tile)
(QB_SH, QA_SH, QA_UN, QC_UN, QB_DIR, QA_DIR, QSOFF, QVOFF, QJK) = range(9)
NQ = 9
# cvec columns: 0..8 coeffs for Q cols, 9 = const, 10 = w_nsq_sh, 11 = w_nsq_v
NCV = 12

# weight-pack column offsets in the [128, 6144] packed weight tensor
_WOFF = {}
_off = 0
for _nm, _kt, _cols in [("w1c", 4, D), ("w2", 2, D), ("wd1", 2, 2 * D),
                        ("wd2", 4, 2 * D), ("wv1c", 4, D), ("wv2", 2, D)]:
    _WOFF[_nm] = (_off, _kt, _cols)
    _off += _kt * _cols
WCOLS = _off  # 6144


# ============================ host-side prep ================================

def _assign_fragments(fid):
    counts = np.bincount(fid, minlength=NF)
    order = np.argsort(-counts, kind="stable")
    core_frags = [[] for _ in range(NCORES)]
    for i, f in enumerate(order):
        r = i // NCORES
        c = i % NCORES if r % 2 == 0 else NCORES - 1 - (i % NCORES)
        core_frags[c].append(int(f))
    layout = []
    for c in range(NCORES):
        frags = sorted(core_frags[c], key=lambda f: -counts[f])
        strips = [[] for _ in range(NSTRIP)]
        fill = np.zeros(NSTRIP, dtype=int)
        for f in frags:
            for s in range(NSTRIP):
                if fill[s] + counts[f] <= P:
                    strips[s].append(f)
                    fill[s] += counts[f]
                    break
            else:
                raise AssertionError(f"core {c}: fragment {f} does not fit")
        layout.append(strips)
    return layout, counts


def _build_core_meta(fid, atom_types, layout, counts):
    nodes_of = {f: np.nonzero(fid == f)[0] for f in range(NF)}
    metas = []
    for c in range(NCORES):
        slot_node = -np.ones(M, dtype=np.int64)
        slot_frag = -np.ones(M, dtype=np.int64)
        frag_global = []
        fl = 0
        for b in range(NSTRIP):
            pos = b * P
            for f in layout[c][b]:
                nn = nodes_of[f]
                slot_node[pos:pos + len(nn)] = nn
                slot_frag[pos:pos + len(nn)] = fl
                frag_global.append(f)
                pos += len(nn)
                fl += 1
        assert fl == NFC
        frag_global = np.array(frag_global, dtype=np.int64)
        real = slot_node >= 0

        pm3 = np.zeros((NSTRIP, P, 3 * P), dtype=np.float32)
        tm = np.zeros((NSTRIP, P, P), dtype=np.float32)
        amat = np.zeros((NSTRIP, P, NFC), dtype=np.float32)
        t2pm_frag = np.zeros(NFC, dtype=np.float32)
        for b in range(NSTRIP):
            sf = slot_frag[b * P:(b + 1) * P]
            sn = slot_node[b * P:(b + 1) * P]
            rr = sf >= 0
            same = (sf[:, None] == sf[None, :]) & rr[:, None] & rr[None, :]
            upper = np.triu(np.ones((P, P), dtype=bool), k=1)
            pmb = (same & upper).astype(np.float32)
            pm3[b] = np.tile(pmb, (1, 3))
            at = np.where(rr, atom_types[np.where(rr, sn, 0)], -1)
            tgt = np.where(at[:, None] == at[None, :], 0.3, 0.1).astype(np.float32)
            tm[b] = tgt * pmb
            for p in range(P):
                if rr[p]:
                    amat[b, p, sf[p]] = 1.0
            for fl_ in np.unique(sf[rr]):
                sel = sf == fl_
                t2pm_frag[fl_] += float(((tgt * pmb) ** 2)[np.ix_(sel, sel)].sum())

        cnt = counts[frag_global].astype(np.float32)
        valid = (cnt >= 2.0).astype(np.float32)
        pairs = cnt * (cnt - 1.0) * 0.5
        safe_c = np.maximum(cnt, 1.0)
        safe_p = np.maximum(pairs, 1.0)

        t_sh = np.float32(BRICS)
        t_dir = np.float32(BRICS - 0.2)
        cvec = np.zeros((NFC, NCV), dtype=np.float32)
        cvec[:, QB_SH] = -2.0 * t_sh * AW / safe_p
        cvec[:, QA_SH] = AW / safe_p
        cvec[:, QA_UN] = (1.0 - AW) / safe_p
        cvec[:, QC_UN] = -2.0 * (1.0 - AW) / safe_p
        cvec[:, QB_DIR] = -2.0 * t_dir * VW / (3.0 * safe_p)
        cvec[:, QA_DIR] = VW / (3.0 * safe_p)
        cvec[:, QSOFF] = -2.0 * AW / (safe_c * safe_c)
        cvec[:, QVOFF] = -2.0 * VW / (safe_c * safe_c)
        cvec[:, QJK] = 0.0
        cvec[:, 9] = (AW * t_sh * t_sh * pairs / safe_p
                      + (1.0 - AW) * t2pm_frag / safe_p
                      + VW * t_dir * t_dir * pairs / safe_p)
        cvec[:, 10] = AW * (1.0 / safe_c - 1.0 / (safe_c * safe_c))
        cvec[:, 11] = VW * (1.0 / safe_c - 1.0 / (safe_c * safe_c))
        cvec[:, 0:10] *= valid[:, None]
        cvec[:, 10:12] *= valid[:, None]

        metas.append(dict(slot_node=slot_node, real=real, pm3=pm3, tm=tm,
                          amat=amat, cvec=cvec))
    return metas


def _pack_rows(x, kt):
    # [kt*128, cols] -> [128, kt*cols] so each partition's data is contiguous
    cols = x.shape[1]
    return np.ascontiguousarray(
        x.reshape(kt, P, cols).transpose(1, 0, 2).reshape(P, kt * cols))


def _shard_inputs(inputs):
    fid = np.asarray(inputs["fragment_ids"]).astype(np.int64)
    at = np.asarray(inputs["atom_types"]).astype(np.int64)
    layout, counts = _assign_fragments(fid)
    metas = _build_core_meta(fid, at, layout, counts)

    W1 = np.asarray(inputs["W1"], np.float32)
    W2 = np.asarray(inputs["W2"], np.float32)
    Wd1 = np.asarray(inputs["Wd1"], np.float32)
    Wd2 = np.asarray(inputs["Wd2"], np.float32)
    Wv1 = np.asarray(inputs["Wv1"], np.float32)
    Wv2 = np.asarray(inputs["Wv2"], np.float32)
    w1c = np.concatenate([R * W1, (1.0 - R) * W1], axis=0)
    wv1c = np.concatenate([R * Wv1, (1.0 - R) * Wv1], axis=0)
    wall = np.concatenate([
        _pack_rows(w1c, 4), _pack_rows(W2, 2), _pack_rows(Wd1, 2),
        _pack_rows(Wd2, 4), _pack_rows(wv1c, 4), _pack_rows(Wv2, 2)],
        axis=1).astype(BF)
    assert wall.shape == (P, WCOLS)

    b2p = (np.asarray(inputs["b2"], np.float32) - LN2 * W2.sum(axis=0))
    bd2p = (np.asarray(inputs["bd2"], np.float32) - LN2 * Wd2.sum(axis=0))
    bv2p = (-LN2 * Wv2.sum(axis=0))
    ball = np.concatenate([
        np.asarray(inputs["b1"], np.float32).reshape(2, P).T,
        b2p.reshape(2, P).T,
        np.asarray(inputs["bd1"], np.float32).reshape(4, P).T,
        bd2p.reshape(4, P).T,
        bv2p.reshape(2, P).T], axis=1).astype(np.float32)
    assert ball.shape == (P, 14)

    ss = np.asarray(inputs["scalar_short"], np.float32)
    sl = np.asarray(inputs["scalar_long"], np.float32)
    vs = np.asarray(inputs["vector_short"], np.float32)
    vl = np.asarray(inputs["vector_long"], np.float32)

    in_maps = []
    for c in range(NCORES):
        m = metas[c]
        idx = np.where(m["real"], m["slot_node"], 0)

        def take2(x):
            g = (x[idx] * m["real"][:, None]).astype(np.float32)
            return _pack_rows(np.ascontiguousarray(g.T), 2).astype(BF)

        def take3(x):
            g = (x[idx] * m["real"][:, None, None]).astype(np.float32)
            t = np.ascontiguousarray(g.transpose(1, 2, 0).reshape(3 * D, M))
            return _pack_rows(t, 6).astype(BF)

        in_maps.append({
            "x_s": take2(ss), "x_l": take2(sl),
            "v_s": take3(vs), "v_l": take3(vl),
            "wall": wall, "ball": ball,
            "pm3": np.ascontiguousarray(
                m["pm3"].transpose(1, 0, 2).reshape(P, NSTRIP * 3 * P)).astype(BF),
            "tm": np.ascontiguousarray(
                m["tm"].transpose(1, 0, 2).reshape(P, NSTRIP * P)).astype(BF),
            "amat": np.ascontiguousarray(
                m["amat"].transpose(1, 0, 2).reshape(P, NSTRIP * NFC)),
            "cvec": m["cvec"],
            "ident": np.ascontiguousarray(np.eye(P, dtype=np.float32)).astype(BF),
        })
    n_valid = float((counts >= 2).sum())
    return in_maps, n_valid


def _combine(fins, n_valid):
    loss = float(sum(float(f[0]) for f in fins))
    ssq_sh = float(sum(float(f[1]) for f in fins))
    ssq_un = float(sum(float(f[2]) for f in fins))
    l2 = 0.01 * (np.sqrt(ssq_sh) + np.sqrt(ssq_un))
    if n_valid > 0:
        return np.float32(CF * (loss + n_valid * l2) / max(n_valid, 1.0))
    return np.float32(0.0)


# ============================ device program ================================

_NC_CACHE = {}


def build_nc(debug_out=False):
    key = ("nc", debug_out)
    if key in _NC_CACHE:
        return _NC_CACHE[key]
    import concourse.bass as bass
    import concourse.bacc as bacc
    import concourse.mybir as mybir
    import concourse.tile as tile

    F32 = mybir.dt.float32
    BF16 = mybir.dt.bfloat16
    AF = mybir.ActivationFunctionType
    ALU = mybir.AluOpType

    nc = bacc.Bacc("TRN2", target_bir_lowering=False, debug=False)

    d = {}
    for name, shape, dt_ in [
        ("x_s", [P, 2 * M], BF16), ("x_l", [P, 2 * M], BF16),
        ("v_s", [P, 6 * M], BF16), ("v_l", [P, 6 * M], BF16),
        ("wall", [P, WCOLS], BF16), ("ball", [P, 14], F32),
        ("pm3", [P, NSTRIP * 3 * P], BF16), ("tm", [P, NSTRIP * P], BF16),
        ("amat", [P, NSTRIP * NFC], F32), ("cvec", [NFC, NCV], F32),
        ("ident", [P, P], BF16),
    ]:
        d[name] = nc.dram_tensor(name, shape, dt_, kind="ExternalInput").ap()
    d_out = nc.dram_tensor("out", [1, 3], F32, kind="ExternalOutput").ap()
    d_inv = nc.dram_tensor("inv_scratch", [5, M], BF16).ap()
    d_dbg = {}
    if debug_out:
        for nm, shape, dt_ in [("dbg_nsq", [P, 45], F32),
                               ("dbg_seg", [NFC, NQ], F32),
                               ("dbg_segn", [NFC, 5], F32),
                               ("dbg_irow", [1, 5 * M], F32)]:
            d_dbg[nm] = nc.dram_tensor(nm, shape, dt_, kind="ExternalOutput").ap()

    with tile.TileContext(nc) as tc, ExitStack() as ctx:
        wpool = ctx.enter_context(tc.tile_pool(name="w", bufs=1))
        feat = ctx.enter_context(tc.tile_pool(name="feat", bufs=1))
        xin = ctx.enter_context(tc.tile_pool(name="xin", bufs=2))
        small = ctx.enter_context(tc.tile_pool(name="small", bufs=1))
        rowp = ctx.enter_context(tc.tile_pool(name="rowp", bufs=2))
        junkp = ctx.enter_context(tc.tile_pool(name="junk", bufs=3))
        psp = ctx.enter_context(tc.tile_pool(name="ps", bufs=4, space="PSUM"))
        pnsq = ctx.enter_context(tc.tile_pool(name="pnsq", bufs=1, space="PSUM"))
        pseg = ctx.enter_context(tc.tile_pool(name="pseg", bufs=1, space="PSUM"))

        # ---- constants / weights (one DMA each); encoder inputs issue first
        # on the sync queue, G-phase masks go on the scalar queue ----
        def ld(name, shape, dt_, pool=wpool, eng=None):
            t = pool.tile(shape, dt_, tag=name, name=name)
            (eng or nc.sync).dma_start(out=t, in_=d[name])
            return t

        wall_sb = ld("wall", [P, WCOLS], BF16)
        ball_sb = ld("ball", [P, 14], F32)
        xs_sb = ld("x_s", [P, 2 * M], BF16, pool=feat)
        xl_sb = ld("x_l", [P, 2 * M], BF16, pool=feat)
        vs_sb = ld("v_s", [P, 6 * M], BF16, pool=feat)
        vl_sb = ld("v_l", [P, 6 * M], BF16, pool=feat)
        pm3_sb = ld("pm3", [P, NSTRIP * 3 * P], BF16, eng=nc.scalar)
        tm_sb = ld("tm", [P, NSTRIP * P], BF16, eng=nc.scalar)
        amat_sb = ld("amat", [P, NSTRIP * NFC], F32, eng=nc.scalar)
        cvec_sb = ld("cvec", [NFC, NCV], F32, eng=nc.scalar)
        ident_sb = ld("ident", [P, P], BF16, eng=nc.scalar)

        def W(name, k):
            off, kt, cols = _WOFF[name]
            assert k < kt
            return wall_sb[:, off + k * cols: off + (k + 1) * cols]

        def bias(name, m_):
            offs = {"b1": 0, "b2p": 2, "bd1": 4, "bd2p": 8, "bv2p": 12}
            return ball_sb[:, offs[name] + m_: offs[name] + m_ + 1]

        ones_col = wpool.tile([P, 1], BF16, tag="ones_col", name="ones_col")
        nc.vector.memset(ones_col, 1.0)
        ones32 = wpool.tile([NFC, 1], F32, tag="ones32", name="ones32")
        nc.vector.memset(ones32, 1.0)
        eps_sb = wpool.tile([P, 1], F32, tag="eps", name="eps")
        nc.vector.memset(eps_sb, 1e-20)

        # ---- persistent per-chunk feature tiles ----
        # sets order: 0=sh 1=un 2=v0 3=v1 4=v2 ; each set has k-tiles 0,1
        u_t = [[[feat.tile([P, CH], BF16, tag=f"u{s}_{k}_{c}", name=f"u{s}_{k}_{c}")
                 for c in range(NCH)] for k in range(2)] for s in range(5)]
        n_t = [[[feat.tile([P, CH], BF16, tag=f"n{s}_{k}_{c}", name=f"n{s}_{k}_{c}")
                 for c in range(NCH)] for k in range(2)] for s in range(5)]

        nsq_ps = pnsq.tile([P, 45], F32, tag="nsq", name="nsq_ps")

        def mm_chunk(pt, w_name, rhs_list, m_):
            kt = len(rhs_list)
            for k in range(kt):
                nc.tensor.matmul(pt[:, 0:CH], W(w_name, k)[:, m_ * P:(m_ + 1) * P],
                                 rhs_list[k], start=(k == 0), stop=(k == kt - 1))

        def xsl(t, k, c):
            return t[:, k * M + c * CH: k * M + (c + 1) * CH]

        # ---- encoder phase A: h1 = softplus(x@W1c + b1), split Exp/Ln ----
        e_h1, t_h1, s_all = [], [], []
        for c in range(NCH):
            ew = xin.tile([P, 2 * CH], BF16, tag="eh1", name=f"eh1_{c}", bufs=2)
            for m_ in range(2):
                pt = psp.tile([P, 512], F32, tag="ps", name="ps")
                mm_chunk(pt, "w1c", [xsl(xs_sb, 0, c), xsl(xs_sb, 1, c),
                                     xsl(xl_sb, 0, c), xsl(xl_sb, 1, c)], m_)
                nc.scalar.activation(ew[:, m_ * CH:(m_ + 1) * CH], pt[:, 0:CH],
                                     AF.Exp, bias=bias("b1", m_))
            e_h1.append(ew)
        for c in range(NCH):
            tw = xin.tile([P, 2 * CH], BF16, tag="th1", name=f"th1_{c}", bufs=2)
            nc.scalar.activation(tw, e_h1[c], AF.Ln, bias=1.0)
            t_h1.append(tw)
        for c in range(NCH):
            s_ = []
            for m_ in range(2):
                pt = psp.tile([P, 512], F32, tag="ps", name="ps")
                mm_chunk(pt, "w2", [t_h1[c][:, 0:CH], t_h1[c][:, CH:2 * CH]], m_)
                t = xin.tile([P, CH], BF16, tag=f"s{m_}", name=f"s{m_}_{c}", bufs=2)
                nc.vector.tensor_scalar_add(t, pt[:, 0:CH], bias("b2p", m_))
                s_.append(t)
            s_all.append(s_)

        # ---- encoder phase B: hd + v1 exps, wide Ln, then dec/v2 evacs ----
        e_B, t_B = [], []
        for c in range(NCH):
            ew = xin.tile([P, 10 * CH], BF16, tag="eB", name=f"eB_{c}", bufs=2)
            for m_ in range(4):
                pt = psp.tile([P, 512], F32, tag="ps", name="ps")
                mm_chunk(pt, "wd1", s_all[c], m_)
                nc.scalar.activation(ew[:, m_ * CH:(m_ + 1) * CH], pt[:, 0:CH],
                                     AF.Exp, bias=bias("bd1", m_))
            for dd in range(3):
                rhs = [xsl(vs_sb, 2 * dd, c), xsl(vs_sb, 2 * dd + 1, c),
                       xsl(vl_sb, 2 * dd, c), xsl(vl_sb, 2 * dd + 1, c)]
                for m_ in range(2):
                    pt = psp.tile([P, 512], F32, tag="ps", name="ps")
                    mm_chunk(pt, "wv1c", rhs, m_)
                    j = 4 + 2 * dd + m_
                    nc.scalar.activation(ew[:, j * CH:(j + 1) * CH], pt[:, 0:CH],
                                         AF.Exp)
            e_B.append(ew)
        for c in range(NCH):
            tw = xin.tile([P, 10 * CH], BF16, tag="tB", name=f"tB_{c}", bufs=2)
            nc.scalar.activation(tw, e_B[c], AF.Ln, bias=1.0)
            t_B.append(tw)

        xsq_t = [[None] * 2 for _ in range(5)]

        def sq_and_nsq(s, k, c, src):
            # xsq = src^2; the one-column matmuls that reduce it to nsq are
            # emitted after the chunk's evacs so each column's two-matmul
            # accumulation group is contiguous
            t = feat.tile([P, CH], BF16, tag=f"xsq{s}_{k}", name=f"xsq{s}_{k}")
            if s < 2:
                nc.gpsimd.tensor_mul(t, src, src)
            else:
                nc.vector.tensor_mul(t, src, src)
            xsq_t[s][k] = t

        def nsq_mms(c):
            for s in range(5):
                for ls in range(3):
                    col = s * NSTRIP + 3 * c + ls
                    for k in range(2):
                        nc.tensor.matmul(
                            nsq_ps[:, col:col + 1],
                            xsq_t[s][k][:, ls * P:(ls + 1) * P], ones_col,
                            start=(k == 0), stop=(k == 1))

        for c in range(NCH):
            hd = [t_B[c][:, m_ * CH:(m_ + 1) * CH] for m_ in range(4)]
            for m_ in range(4):
                pt = psp.tile([P, 512], F32, tag="ps", name="ps")
                mm_chunk(pt, "wd2", hd, m_)
                s, k = (0, m_) if m_ < 2 else (1, m_ - 2)
                nc.scalar.activation(u_t[s][k][c], pt[:, 0:CH], AF.Identity,
                                     bias=bias("bd2p", m_))
                sq_and_nsq(s, k, c, u_t[s][k][c])
            for dd in range(3):
                v1 = [t_B[c][:, (4 + 2 * dd + k) * CH:(5 + 2 * dd + k) * CH]
                      for k in range(2)]
                for m_ in range(2):
                    pt = psp.tile([P, 512], F32, tag="ps", name="ps")
                    mm_chunk(pt, "wv2", v1, m_)
                    nc.vector.tensor_scalar_add(u_t[2 + dd][m_][c], pt[:, 0:CH],
                                                bias("bv2p", m_))
                    sq_and_nsq(2 + dd, m_, c, u_t[2 + dd][m_][c])
            nsq_mms(c)

        # ---- inverse norms: inv = (nsq+eps)^(-1/2); transpose on the tensor
        # engine to [(s b), p] so the DRAM bounce is two contiguous DMAs ----
        nsq_sb = small.tile([P, 45], F32, tag="nsq_sb", name="nsq_sb")
        nc.vector.tensor_copy(nsq_sb, nsq_ps)
        lnn = small.tile([P, 45], F32, tag="lnn", name="lnn")
        nc.scalar.activation(lnn, nsq_ps, AF.Ln, bias=eps_sb[:, 0:1])
        invN = small.tile([P, 45], BF16, tag="invN", name="invN")
        nc.scalar.activation(invN, lnn, AF.Exp, scale=-0.5)
        invT_ps = pseg.tile([45, P], BF16, tag="invT", name="invT")
        nc.tensor.transpose(invT_ps, invN, ident_sb)
        invT = small.tile([45, P], BF16, tag="invT_sb", name="invT_sb")
        nc.vector.tensor_copy(invT, invT_ps)
        nc.sync.dma_start(out=d_inv.rearrange("s (b p) -> (s b) p", p=P),
                          in_=invT)
        irow = rowp.tile([1, 5 * M], BF16, tag="irow", name="irow", bufs=1)
        nc.sync.dma_start(out=irow,
                          in_=d_inv.rearrange("(o s) m -> o (s m)", o=1))

        # ---- normalize: B_s = bcast(inv row) on GpSimd; n = u * B on DVE ----
        for s in range(5):
            B_s = rowp.tile([P, M], BF16, tag="Bb", name=f"B{s}", bufs=2)
            nc.gpsimd.partition_broadcast(B_s, irow[0:1, s * M:(s + 1) * M])
            for c in range(NCH):
                for k in range(2):
                    nc.vector.tensor_mul(n_t[s][k][c], u_t[s][k][c],
                                         B_s[:, c * CH:(c + 1) * CH])

        # ---- G phase: per-strip grams + masked pair reductions ----
        def msum(out_t, in0, in1, acc):
            nc.vector.scalar_tensor_tensor(
                out=out_t, in0=in0, scalar=1.0, in1=in1,
                op0=ALU.bypass, op1=ALU.mult, accum_out=acc)

        Q_b = []
        for b in range(NSTRIP):
            cb, ls = b // 3, (b % 3)
            lsl = slice(ls * P, (ls + 1) * P)
            pmb = pm3_sb[:, b * 3 * P: b * 3 * P + P]
            pm3b = pm3_sb[:, b * 3 * P: (b + 1) * 3 * P]
            tmb = tm_sb[:, b * P:(b + 1) * P]
            Q = small.tile([P, NQ], F32, tag=f"Q{b}", name=f"Q{b}")
            Q_b.append(Q)

            def gpair(pt, col0, tiles):
                for k in range(2):
                    nc.tensor.matmul(pt[:, col0:col0 + P], tiles[k][cb][:, lsl],
                                     tiles[k][cb][:, lsl],
                                     start=(k == 0), stop=(k == 1))

            # unnormalized grams (sh + 3 v dims) packed in one bank
            gu = psp.tile([P, 512], F32, tag="ps", name="gu")
            gpair(gu, 0, u_t[0])
            for dd in range(3):
                gpair(gu, P + dd * P, u_t[2 + dd])
            j5 = junkp.tile([P, P], BF16, tag="jk1", name="j5")
            msum(j5, gu[:, 0:P], pmb, Q[:, QSOFF:QSOFF + 1])
            j6 = junkp.tile([P, 3 * P], BF16, tag="jk3", name="j6")
            msum(j6, gu[:, P:4 * P], pm3b, Q[:, QVOFF:QVOFF + 1])

            # normalized grams: [sh | un] bank and [v0 | v1 | v2] bank
            gn = psp.tile([P, 512], F32, tag="ps", name="gn")
            gpair(gn, 0, n_t[0])
            gpair(gn, P, n_t[1])
            gv = psp.tile([P, 512], F32, tag="ps", name="gv")
            for dd in range(3):
                gpair(gv, dd * P, n_t[2 + dd])

            spm = junkp.tile([P, P], BF16, tag="spm", name="spm", bufs=2)
            msum(spm, gn[:, 0:P], pmb, Q[:, QB_SH:QB_SH + 1])
            nc.scalar.activation(junkp.tile([P, P], BF16, tag="jk1", name="j1"),
                                 spm, AF.Square, accum_out=Q[:, QA_SH:QA_SH + 1])
            upm = junkp.tile([P, P], BF16, tag="upm", name="upm", bufs=2)
            msum(upm, gn[:, P:2 * P], pmb, Q[:, QJK:QJK + 1])
            nc.scalar.activation(junkp.tile([P, P], BF16, tag="jk1", name="j2"),
                                 upm, AF.Square, accum_out=Q[:, QA_UN:QA_UN + 1])
            j3 = junkp.tile([P, P], BF16, tag="jk1", name="j3")
            msum(j3, gn[:, P:2 * P], tmb, Q[:, QC_UN:QC_UN + 1])

            dpm = junkp.tile([P, 3 * P], BF16, tag="dpm", name="dpm", bufs=2)
            msum(dpm, gv[:, 0:3 * P], pm3b, Q[:, QB_DIR:QB_DIR + 1])
            nc.scalar.activation(junkp.tile([P, 3 * P], BF16, tag="jk3", name="j4"),
                                 dpm, AF.Square, accum_out=Q[:, QA_DIR:QA_DIR + 1])

        # ---- segment reduction + final combine ----
        segq_ps = pseg.tile([NFC, NQ + 5], F32, tag="seg", name="seg")
        seg_ps = segq_ps[:, 0:NQ]
        segn_ps = segq_ps[:, NQ:NQ + 5]
        nsq_v = nsq_sb.rearrange("p (s b) -> p s b", s=5)
        for b in range(NSTRIP):
            nc.tensor.matmul(seg_ps, amat_sb[:, b * NFC:(b + 1) * NFC], Q_b[b],
                             start=(b == 0), stop=(b == NSTRIP - 1))
        for b in range(NSTRIP):
            nc.tensor.matmul(segn_ps, amat_sb[:, b * NFC:(b + 1) * NFC],
                             nsq_v[:, :, b],
                             start=(b == 0), stop=(b == NSTRIP - 1))
        segq = small.tile([NFC, NQ + 5], F32, tag="segs", name="segs")
        nc.vector.tensor_copy(segq, segq_ps)
        segs = segq[:, 0:NQ]
        segn = segq[:, NQ:NQ + 5]

        acc = small.tile([NFC, 3], F32, tag="acc", name="acc")
        junkq = small.tile([NFC, NQ], F32, tag="junkq", name="junkq")
        # acc0 = sum_q cvec[q]*segs[q]  (rowwise dot via masked accumulate)
        nc.vector.scalar_tensor_tensor(
            out=junkq, in0=segs, scalar=1.0, in1=cvec_sb[:, 0:NQ],
            op0=ALU.bypass, op1=ALU.mult, accum_out=acc[:, 0:1])
        # + const + w_nsq_sh * segn_sh + w_nsq_v * (segn_v0+v1+v2)
        nc.vector.tensor_add(acc[:, 0:1], acc[:, 0:1], cvec_sb[:, 9:10])
        nc.vector.scalar_tensor_tensor(
            out=junkq[:, 0:1], in0=segn[:, 0:1], scalar=cvec_sb[:, 10:11],
            in1=acc[:, 0:1], op0=ALU.mult, op1=ALU.add, accum_out=None)
        nc.vector.tensor_copy(acc[:, 0:1], junkq[:, 0:1])
        vtot = small.tile([NFC, 1], F32, tag="vtot", name="vtot")
        nc.vector.tensor_add(vtot, segn[:, 2:3], segn[:, 3:4])
        nc.vector.tensor_add(vtot, vtot, segn[:, 4:5])
        nc.vector.scalar_tensor_tensor(
            out=junkq[:, 1:2], in0=vtot, scalar=cvec_sb[:, 11:12],
            in1=acc[:, 0:1], op0=ALU.mult, op1=ALU.add, accum_out=None)
        nc.vector.tensor_copy(acc[:, 0:1], junkq[:, 1:2])
        nc.vector.tensor_copy(acc[:, 1:2], segn[:, 0:1])
        nc.vector.tensor_copy(acc[:, 2:3], segn[:, 1:2])

        if debug_out:
            nc.sync.dma_start(out=d_dbg["dbg_nsq"], in_=nsq_sb)
            nc.sync.dma_start(out=d_dbg["dbg_seg"], in_=segs)
            nc.sync.dma_start(out=d_dbg["dbg_segn"], in_=segn)
            irow_f = rowp.tile([1, 5 * M], F32, tag="irow_f", name="irow_f", bufs=1)
            nc.vector.tensor_copy(irow_f, irow)
            nc.sync.dma_start(out=d_dbg["dbg_irow"], in_=irow_f)

        fin_ps = pseg.tile([1, 3], F32, tag="fin", name="fin")
        nc.tensor.matmul(fin_ps, ones32, acc, start=True, stop=True)
        fin_sb = small.tile([1, 3], F32, tag="fin_sb", name="fin_sb")
        nc.vector.tensor_copy(fin_sb, fin_ps)
        nc.sync.dma_start(out=d_out, in_=fin_sb)

    nc.compile()
    _NC_CACHE[key] = nc
    return nc


# ============================== entry point =================================

def kernel(**inputs) -> np.ndarray:
    from concourse.bass_utils import run_bass_kernel_spmd

    in_maps, n_valid = _shard_inputs(inputs)
    nc = build_nc()
    res = run_bass_kernel_spmd(nc, in_maps, core_ids=list(range(NCORES)))
    fins = [r["out"].reshape(3) for r in res.results]
    return _combine(fins, n_valid)
